# revision 26
# baseline (speedup 1.0000x reference)
"""Multi-head attention (B=2, S=2048, D=1024, H=16, causal + key-pad mask)
as an 8-core Trainium2 Bass/Tile SPMD kernel.

Sharding: data parallel over the 2 batches (4 cores each); within a batch
group, tensor parallel over heads (4 heads/core) for the QKV projections and
attention. Head outputs are softmax-normalized on the owning core, cast to
bf16 and AllGathered per 512-wide q-tile; the O-projection is column-sliced
(each core computes its own 256 output columns for ALL rows) and pipelined
per q-tile into the attention stream, so only the last AllGather plus one
small O-chunk sits on the serial tail.

All matmul operands are bf16 (fp32 PSUM accumulation): bf16 streams at
1 cyc/row on the PE where f32r measured ~2.7, and it halves HBM/SBUF/wire
traffic. The key-pad mask is folded into the V projection (masked key rows
and their denominator 'ones' column are zeroed), so exp needs no bias.
Causal masking: block-level loop bounds + a -1e9 triangular DVE add on
diagonal blocks. Warm-up and keep-alive matmuls prevent the PE HAM clock
from dropping to half rate during DMA/collective-only windows.

self-contained: includes a workaround for the walrus per-instruction
sync-wait limit and an NTFF-profile hook shim.
"""
import sys
import types

import numpy as np
import ml_dtypes

import bass_rust
import concourse.bass as bass
import concourse.mybir as mybir
import concourse.tile as tile


# ---- walrus sync-wait limit workaround ----------------------------------
# This walrus build rejects instructions carrying more than one sem wait
# ("Too many sync wait commands"). Tile emits multi-wait instructions (the
# final drain, matmuls waiting on several DMA queues). Split excess waits
# onto same-engine NoOps placed immediately before the instruction --
# serial waits on one sequencer are semantically identical.
_WSPLIT_COUNTER = [0]


def _split_excess_waits(nc, limit=1):
    for fn in nc.m.functions:
        for bb in fn.blocks:
            out = []
            changed = False
            for inst in bb.instructions:
                si = inst.sync_info
                waits = list(si.on_wait) if si is not None and si.on_wait else []
                if len(waits) > limit:
                    extra, keep = waits[:-limit], waits[-limit:]
                    for s in range(0, len(extra), limit):
                        _WSPLIT_COUNTER[0] += 1
                        nop = mybir.InstNoOp(
                            name=f"I-wsplit-{_WSPLIT_COUNTER[0]}", ins=[], outs=[]
                        )
                        nop.engine = inst.engine
                        nop.sync_info = bass_rust.SyncInfo(
                            on_wait=extra[s : s + limit], on_update=[]
                        )
                        out.append(nop)
                    si.on_wait = keep
                    changed = True
                out.append(inst)
            if changed:
                bb.instructions = out


def _install_tile_patch():
    if getattr(tile.TileContext, "_wait_split_patched", False):
        return
    orig_exit = tile.TileContext.__exit__

    def __exit__(self, exc_type, exc_val, exc_tb):
        r = orig_exit(self, exc_type, exc_val, exc_tb)
        if exc_type is None:
            _split_excess_waits(self.nc)
        return r

    tile.TileContext.__exit__ = __exit__
    tile.TileContext._wait_split_patched = True


_install_tile_patch()


# ---- NTFF profile hook shim (axon deployments missing antenv.axon_hooks) --
def _install_ntff_hook():
    try:
        import antenv.axon_hooks  # noqa: F401
        return
    except ImportError:
        pass
    try:
        from trn_agent_boot.trn_boot import _ntff_profile_via_ctypes

        hook = _ntff_profile_via_ctypes("/opt/axon/libaxon_pjrt.so")
    except Exception:
        hook = None
    m = types.ModuleType("antenv.axon_hooks")
    m.get_axon_ntff_profile_hook = lambda: hook
    m.set_axon_ntff_profile_hook = lambda h: None
    sys.modules["antenv.axon_hooks"] = m


_install_ntff_hook()

from concourse.bass_utils import run_bass_kernel_spmd  # noqa: E402

f32 = mybir.dt.float32
f32r = mybir.dt.float32r
bf16 = mybir.dt.bfloat16

B, S, D, H, HD = 2, 2048, 1024, 16, 64
HPC, GROUP = 4, 4          # heads per core, cores per batch
HC = HPC * HD              # 256 projection cols per core
NKT = S // 128             # 16 k-tiles
NQT = S // 512             # 4 q-tiles
QT = 512                   # q-tile width
SCALE = 1.0 / np.sqrt(HD)  # 0.125
NEG = -1.0e9
KCH = D // 128             # 8 contraction chunks

REPLICA_GROUPS = [[0, 1, 2, 3], [4, 5, 6, 7]]

ADD = mybir.AluOpType.add
MULT = mybir.AluOpType.mult
EXP = mybir.ActivationFunctionType.Exp


def r32(ap):
    return ap.bitcast(f32r)


def build(cfg):
    # cfg: dict with NKT_C (compacted k-tiles), NCK (k-proj 512-col units),
    # nk_cs[NQT], bk0s[NQT], MAXBD (mask tiles per q-tile)
    nc = bass.Bass()
    dp = nc.declare_dram_parameter
    SK = cfg["NCK"] * QT
    xT = dp("xT", [D, S], bf16, isOutput=False)
    xTs = dp("xTs", [D, SK], bf16, isOutput=False)     # compacted keys (padded)
    wqT = dp("wqT", [D, HC], bf16, isOutput=False)
    wkT = dp("wkT", [D, HC], bf16, isOutput=False)
    wvT = dp("wvT", [D, HC], bf16, isOutput=False)
    woT = dp("woT", [D, HC], bf16, isOutput=False)     # this core's 256 out cols
    bq = dp("bq", [128, 2], f32, isOutput=False)
    bk = dp("bk", [128, 2], f32, isOutput=False)
    bv = dp("bv", [1, HC], bf16, isOutput=False)
    bo = dp("bo", [128, 2], f32, isOutput=False)       # this core's 256 out cols
    padv = dp("padv", [128, cfg["NKT_C"]], f32, isOutput=False)  # 1 real / 0 pad
    kmask = dp("kmask", [NQT, cfg["MAXBD"], 128, QT], bf16, isOutput=False)
    esel = dp("esel", [H, D], bf16, isOutput=False)     # head-of-column selector
    outT = dp("outT", [HC, S], f32, isOutput=True)

    with tile.TileContext(nc) as tc:
        _body(nc, tc, cfg, locals())
    return nc


def _body(nc, tc, cfg, t):
    xT, wqT, wkT, wvT, woT = t["xT"], t["wqT"], t["wkT"], t["wvT"], t["woT"]
    bq, bk, bv, bo, padv = t["bq"], t["bk"], t["bv"], t["bo"], t["padv"]
    xTs, kmask = t["xTs"], t["kmask"]
    esel = t["esel"]
    outT = t["outT"]
    NKT_C, NCK = cfg["NKT_C"], cfg["NCK"]
    nk_cs, bk0s, MAXBD = cfg["nk_cs"], cfg["bk0s"], cfg["MAXBD"]
    SK = NCK * QT

    ctx_pools = []

    def pool(name, bufs, space="SBUF"):
        p = tc.tile_pool(name=name, bufs=bufs, space=space)
        ctx_pools.append(p)
        return p.__enter__()

    dram_pool = pool("dram", 1, space="DRAM")
    HR = HD + 1  # 65 rows/head: 64 o-rows + softmax denominator
    ag_in = dram_pool.tile([NQT, HPC * HR, QT], bf16)       # [4, 260, 512]
    ag_out = dram_pool.tile([NQT, H * HR, QT], bf16)        # [4, 1040, 512]
    agw_in = dram_pool.tile([64, 64], bf16)
    agw_out = dram_pool.tile([GROUP * 64, 64], bf16)

    const = pool("const", 1)
    probs_pool = pool("probs", 2)
    agsb_pool = pool("agsb", 2)
    stg_pool = pool("stg", 3)
    outsb_pool = pool("outsb", 2)

    ps_st = pool("ps_st", 2, space="PSUM")
    ps_ot = pool("ps_ot", 2, space="PSUM")
    ps_mm = pool("ps_mm", 2, space="PSUM")

    # ---- PE warm-up: ~4us of dependency-free matmuls so the HAM clock ----
    # gate opens while the input DMAs stream in.
    wu_a = const.tile([128, 128], bf16)
    nc.any.memset(wu_a[:], 0.015625)
    wu_b = const.tile([128, QT], bf16)
    nc.any.memset(wu_b[:], 0.015625)

    def warm_mms(n):
        for _ in range(n):
            ps = ps_mm.tile([128, QT], f32, tag="mm")
            nc.tensor.matmul(ps[:], wu_a[:], wu_b[:], start=True, stop=True)

    warm_mms(40)

    # ---- resident inputs -------------------------------------------------
    xt = const.tile([128, KCH, S], bf16)      # xT, chunk-major (queries)
    xts = const.tile([128, KCH, SK], bf16)    # compacted keys, chunk-major
    wq_t = const.tile([128, KCH, HC], bf16)
    wk_t = const.tile([128, KCH, HC], bf16)
    wv_t = const.tile([128, KCH, HC], bf16)
    wo_t = const.tile([128, KCH, HC], bf16)
    bq_t = const.tile([128, 2], f32)
    nc.sync.dma_start(bq_t[:], bq[:])
    bk_t = const.tile([128, 2], f32)
    nc.sync.dma_start(bk_t[:], bk[:])
    bo_t = const.tile([128, 2], f32)
    nc.sync.dma_start(bo_t[:], bo[:])
    padv_t = const.tile([128, NKT_C], f32)
    nc.sync.dma_start(padv_t[:], padv[:])
    km_t = const.tile([128, NQT, MAXBD, QT], bf16)
    for n in range(NQT):
        for m in range(MAXBD):
            nc.scalar.dma_start(km_t[:, n, m, :], kmask[n, m])
    bv_row = const.tile([128, HC], bf16)
    nc.any.memset(bv_row[:], 0.0)
    nc.sync.dma_start(bv_row[0:1, :], bv[:])

    wqT_r = wqT.rearrange("(c p) j -> p c j", p=128)
    wkT_r = wkT.rearrange("(c p) j -> p c j", p=128)
    wvT_r = wvT.rearrange("(c p) j -> p c j", p=128)
    woT_r = woT.rearrange("(c p) j -> p c j", p=128)
    xT_r = xT.rearrange("(c p) s -> p c s", p=128)
    xTs_r = xTs.rearrange("(c p) s -> p c s", p=128)
    # critical path (q-tile 0) on the sync queue; the rest on gpsimd's queue
    for k in range(KCH):
        nc.sync.dma_start(wk_t[:, k], wkT_r[:, k])
        nc.sync.dma_start(wq_t[:, k], wqT_r[:, k])
        nc.sync.dma_start(xt[:, k, 0:QT], xT_r[:, k, 0:QT])
        nc.sync.dma_start(xts[:, k, 0:QT], xTs_r[:, k, 0:QT])
        nc.sync.dma_start(wv_t[:, k], wvT_r[:, k])
    # warm-up AllGather first on the gpsimd queue: absorbs the TOPSP cold
    # start during the input DMA phase so the first real AllGather runs fast
    nc.gpsimd.dma_start(agw_in[:], wu_a[0:64, 0:64])
    nc.gpsimd.collective_compute(
        "AllGather",
        mybir.AluOpType.bypass,
        replica_groups=REPLICA_GROUPS,
        ins=[agw_in[:]],
        outs=[agw_out[:]],
    )
    for c in range(1, NQT):
        for k in range(KCH):
            nc.sync.dma_start(
                xt[:, k, c * QT : (c + 1) * QT], xT_r[:, k, c * QT : (c + 1) * QT]
            )
    for c in range(1, NCK):
        for k in range(KCH):
            nc.sync.dma_start(
                xts[:, k, c * QT : (c + 1) * QT], xTs_r[:, k, c * QT : (c + 1) * QT]
            )
    for k in range(KCH):
        nc.gpsimd.dma_start(wo_t[:, k], woT_r[:, k])

    # e0: row 0 = ones, rest 0 -- broadcast-matmul stationary
    e0h = const.tile([128, 128], bf16)
    nc.any.memset(e0h[:], 0.0)
    ones_row_h = const.tile([1, 128], bf16)
    nc.any.memset(ones_row_h[:], 1.0)
    nc.vector.tensor_copy(e0h[0:1, :], ones_row_h[0:1, :])

    # bvb: bv broadcast to all 128 partitions (e0h row 0 is ones, rest 0)
    bvb = const.tile([128, HC], f32)
    bv_ps = ps_mm.tile([128, HC], f32, tag="mm")
    nc.tensor.matmul(bv_ps[:], e0h[:], bv_row[:], start=True, stop=True)
    nc.vector.tensor_copy(bvb[:], bv_ps[:])

    # esel stationary (rows 16-127 zero) and reciprocal-broadcast tiles
    esel_t = const.tile([128, KCH, 128], bf16)
    nc.any.memset(esel_t[:], 0.0)
    nc.sync.dma_start(
        esel_t[0:H], esel.rearrange("h (c n) -> h c n", n=128)
    )
    rec16_a = const.tile([128, QT], bf16)
    nc.any.memset(rec16_a[:], 0.0)
    rec16_b = const.tile([128, QT], bf16)
    nc.any.memset(rec16_b[:], 0.0)
    rec16s = (rec16_a, rec16_b)

    # projection outputs
    qh_t = const.tile([128, 2, S], bf16)    # [j-in-tile, j-tile(pair), s]
    kh_t = const.tile([128, 2, SK], bf16)
    vh_t = const.tile([128, NKT_C, HPC, HD + 1], bf16)
    # denominator 'ones' column: 1.0 for real keys, 0.0 for padding
    nc.vector.tensor_copy(
        vh_t[:, :, :, HD : HD + 1],
        padv_t[:].rearrange("p (t u v) -> p t u v", u=1, v=1).broadcast_to(
            [128, NKT_C, HPC, 1]
        ),
    )

    def proj_qk(w_t, b_t, out_t, jt, c, x_src):
        # one [128, 512] tile of qhT/khT: out partition=j, free=s
        ps = ps_mm.tile([128, QT], f32, tag="mm")
        for k in range(KCH):
            nc.tensor.matmul(
                ps[:],
                w_t[:, k, jt * 128 : (jt + 1) * 128],
                x_src[:, k, c * QT : (c + 1) * QT],
                start=(k == 0),
                stop=(k == KCH - 1),
            )
            if k % 2 == 1:
                yield
        nc.vector.tensor_scalar_add(
            out_t[:, jt, c * QT : (c + 1) * QT], ps[:], b_t[:, jt : jt + 1]
        )

    def proj_v(st_):
        """one s-tile of vh: out partition=s, free=[4 heads x 64]; masked rows zeroed."""
        ps = ps_mm.tile([128, HC], f32, tag="mm")
        for k in range(KCH):
            nc.tensor.matmul(
                ps[:],
                xts[:, k, st_ * 128 : (st_ + 1) * 128],
                wv_t[:, k, :],
                start=(k == 0),
                stop=(k == KCH - 1),
            )
            if k % 2 == 1:
                yield
        vsl = vh_t[:, st_, :, 0:HD]
        nc.vector.tensor_tensor(
            vsl,
            ps[:].rearrange("p (h d) -> p h d", h=HPC),
            bvb[:].rearrange("p (h d) -> p h d", h=HPC),
            ADD,
        )
        nc.vector.tensor_scalar_mul(vsl, vsl, padv_t[:, st_ : st_ + 1])

    agsb_tiles = {}
    den_tiles = {}

    def norm_recip(qi):
        # one batched reciprocal of all 16 gathered denominators
        den_sb = den_tiles[qi]
        den_f = stg_pool.tile([H, QT], f32, tag="denf")
        nc.vector.tensor_copy(den_f[:], den_sb[:])
        rec_f = stg_pool.tile([H, QT], f32, tag="recf")
        nc.vector.reciprocal(rec_f[:], den_f[:])
        rec16 = rec16s[qi % 2]
        with nc.allow_low_precision(reason="bf16 staging precision"):
            nc.vector.tensor_copy(rec16[0:H, :], rec_f[:])
        yield

    def norm_chunks(qi, cs):
        # broadcast 1/den per head over its 64 rows; scale agsb in place
        agsb = agsb_tiles[qi]
        rec16 = rec16s[qi % 2]
        for c in cs:
            bcst_ps = ps_mm.tile([128, QT], f32, tag="mm")
            nc.tensor.matmul(
                bcst_ps[:], esel_t[:, c, :], rec16[:], start=True, stop=True
            )
            with nc.allow_low_precision(reason="bf16 staging precision"):
                nc.vector.tensor_tensor(
                    agsb[:, c, :], agsb[:, c, :], bcst_ps[:], MULT
                )
            yield

    def o_chunk(qi, jh):
        """O-projection for q-tile qi, output-column half jh (128 j cols)."""
        agsb = agsb_tiles[qi]
        ps = ps_mm.tile([128, QT], f32, tag="mm")
        for dc in range(KCH):
            nc.tensor.matmul(
                ps[:],
                wo_t[:, dc, jh * 128 : (jh + 1) * 128],
                agsb[:, dc, :],
                start=(dc == 0),
                stop=(dc == KCH - 1),
            )
            if dc % 2 == 1:
                yield
        osb = outsb_pool.tile([128, QT], f32, tag="osb")
        nc.vector.tensor_scalar_add(osb[:], ps[:], bo_t[:, jh : jh + 1])
        nc.sync.dma_start(
            outT[jh * 128 : (jh + 1) * 128, qi * QT : (qi + 1) * QT], osb[:]
        )

    def attention_qtile(qi, filler):
        q0 = qi * QT
        nk = nk_cs[qi]
        bk0 = bk0s[qi]
        for pair in range(2):
            probs = probs_pool.tile([128, NKT_C, 2, QT], bf16, tag="probs")

            def emit_sc(kt):
                k0 = kt * 128
                st = ps_st.tile([128, 2, QT], f32, tag="st")
                for hh in range(2):
                    nc.tensor.matmul(
                        st[:, hh, :],
                        kh_t[hh * 64 : hh * 64 + 64, pair, k0 : k0 + 128],
                        qh_t[hh * 64 : hh * 64 + 64, pair, q0 : q0 + QT],
                        start=True,
                        stop=True,
                    )
                if kt >= bk0:  # causal staircase mask (host-precomputed)
                    nc.vector.tensor_tensor(
                        st[:, :, :],
                        st[:, :, :],
                        km_t[:, qi, kt - bk0, :].rearrange(
                            "p (o n) -> p o n", o=1
                        ).broadcast_to([128, 2, QT]),
                        ADD,
                    )
                nc.scalar.activation(
                    probs[:, kt, :, :],
                    st[:, :, :],
                    EXP,
                    bias=0.0,
                    scale=float(SCALE),
                )

            emit_sc(0)
            ot0 = ps_ot.tile([HD + 1, QT], f32, tag="ot")
            ot1 = ps_ot.tile([HD + 1, QT], f32, tag="ot")
            ots = (ot0, ot1)
            for kt in range(nk):
                if kt + 1 < nk:
                    emit_sc(kt + 1)
                for hh in range(2):
                    h = 2 * pair + hh
                    nc.tensor.matmul(
                        ots[hh][:, :],
                        vh_t[:, kt, h, :],
                        probs[:, kt, hh, :],
                        start=(kt == 0),
                        stop=(kt == nk - 1),
                    )
                filler()
            # stage unnormalized o (rows 0-255) + denominator (rows 256-259)
            for hh in range(2):
                h = 2 * pair + hh
                stg = stg_pool.tile([HD + 1, QT], bf16, tag="stg")
                nc.vector.tensor_copy(stg[:], ots[hh][:])
                nc.sync.dma_start(
                    ag_in[qi, h * HD : (h + 1) * HD, :], stg[0:HD, :]
                )
                nc.sync.dma_start(
                    ag_in[qi, HPC * HD + h : HPC * HD + h + 1, :], stg[HD : HD + 1, :]
                )

    # ---- emission: projections + O-chunks finely interleaved -------------
    def kneed(qi):
        return min(NCK, (nk_cs[qi] * 128 + QT - 1) // QT)

    emitted = {"kc": 0, "qc": 0, "vst": 0}

    def proj_units_for(qi):
        # units that must complete before attention(qi) runs
        units = []
        for c in range(emitted["kc"], kneed(qi)):
            for jt in range(2):
                units.append(
                    lambda jt=jt, c=c: proj_qk(wk_t, bk_t, kh_t, jt, c, xts)
                )
        emitted["kc"] = max(emitted["kc"], kneed(qi))
        for c in range(emitted["qc"], qi + 1):
            for jt in range(2):
                units.append(
                    lambda jt=jt, c=c: proj_qk(wq_t, bq_t, qh_t, jt, c, xt)
                )
        emitted["qc"] = max(emitted["qc"], qi + 1)
        for st_ in range(emitted["vst"], nk_cs[qi]):
            units.append(lambda st_=st_: proj_v(st_))
        emitted["vst"] = max(emitted["vst"], nk_cs[qi])
        return units

    def o_chunk_units(qi):
        return (
            [lambda qi=qi: norm_recip(qi)]
            + [
                lambda qi=qi, cs=cs: norm_chunks(qi, cs)
                for cs in ((0, 1, 2, 3), (4, 5, 6, 7))
            ]
            + [lambda jh=jh, qi=qi: o_chunk(qi, jh) for jh in range(2)]
        )

    class Filler:
        def __init__(self, units, budget):
            self.units = list(units)
            self.gen = None
            self.budget = budget

        def __call__(self):
            for _ in range(self.budget):
                if self.gen is None:
                    if not self.units:
                        return
                    self.gen = self.units.pop(0)()
                try:
                    next(self.gen)
                except StopIteration:
                    self.gen = None

        def flush(self):
            while self.units or self.gen is not None:
                if self.gen is None:
                    self.gen = self.units.pop(0)()
                for _ in self.gen:
                    pass
                self.gen = None

    Filler(proj_units_for(0), 1).flush()
    for qi in range(NQT):
        pending = proj_units_for(qi + 1) if qi + 1 < NQT else []
        n_att = 2 * nk_cs[qi]             # filler() call sites this q-tile
        total_steps = len(pending) * 5
        budget = max(1, (total_steps + n_att - 1) // n_att)
        filler = Filler(pending, budget)
        attention_qtile(qi, filler)
        filler.flush()
        # AllGather this q-tile's head outputs across the group
        nc.gpsimd.collective_compute(
            "AllGather",
            mybir.AluOpType.bypass,
            replica_groups=REPLICA_GROUPS,
            ins=[ag_in[qi]],
            outs=[ag_out[qi]],
        )
        agsb = agsb_pool.tile([128, KCH, QT], bf16, tag="agsb")
        agsb_tiles[qi] = agsb
        den_sb = agsb_pool.tile([H, QT], bf16, tag="den")
        den_tiles[qi] = den_sb
        RB = HPC * HR  # 260 rows per rank block
        for r in range(GROUP):
            nc.gpsimd.dma_start(
                agsb[:, 2 * r : 2 * r + 2, :],
                ag_out[qi, r * RB : r * RB + HPC * HD, :].rearrange(
                    "(c p) q -> p c q", p=128
                ),
            )
            nc.gpsimd.dma_start(
                den_sb[HPC * r : HPC * r + HPC, :],
                ag_out[qi, r * RB + HPC * HD : (r + 1) * RB, :],
            )
        # normalize + O-project q-tile qi-2 (its AllGather finished two
        # attention spans ago) while AG(qi) is on the wire
        if qi >= 2:
            Filler(o_chunk_units(qi - 2), 1).flush()

    # tail: O-chunk for q-tile 2, keep the PE clock warm while the last
    # AllGather drains, then the final O-chunk
    Filler(o_chunk_units(NQT - 2), 1).flush()
    warm_mms(110)
    Filler(o_chunk_units(NQT - 1), 1).flush()

    for p in reversed(ctx_pools):
        p.__exit__(None, None, None)


# ---- host-side marshalling ----------------------------------------------


def compact_cfg(pad_mask):
    """Key-compaction geometry shared by both batches (max-padded)."""
    pad_mask = np.asarray(pad_mask)
    sels = [np.where(~pad_mask[b])[0] for b in range(B)]

    def cnt(b, p):
        return int(np.searchsorted(sels[b], p))

    nk_cs, bk0s = [], []
    for qi in range(NQT):
        q0 = qi * QT
        c_end = max(cnt(b, q0 + QT) for b in range(B))
        nk_cs.append(max(1, -(-c_end // 128)))
        bk0s.append(min(cnt(b, q0) // 128 for b in range(B)))
    return {
        "NKT_C": nk_cs[-1],
        "NCK": -(-(nk_cs[-1] * 128) // QT),
        "nk_cs": nk_cs,
        "bk0s": bk0s,
        "MAXBD": max(nk_cs[i] - bk0s[i] for i in range(NQT)),
        "sels": sels,
    }


def make_inputs(q, pad_mask, Wq, bq, Wk, bk, Wv, bv, Wo, bo):
    """Build the 8 per-core input maps from full inputs."""
    bf = ml_dtypes.bfloat16
    cfg = compact_cfg(pad_mask)
    NKT_C, NCK, MAXBD = cfg["NKT_C"], cfg["NCK"], cfg["MAXBD"]
    SK = NCK * QT
    esel_m = np.ascontiguousarray(
        (np.arange(D)[None, :] // HD == np.arange(H)[:, None]).astype(bf)
    )
    in_maps = []
    xTs_full = [np.ascontiguousarray(q[b].T).astype(bf) for b in range(B)]
    xTs_sel, padvs, kmasks = [], [], []
    for b in range(B):
        sel = cfg["sels"][b]
        n_sel = len(sel)
        xs = np.zeros((SK, D), dtype=np.float32)
        xs[:n_sel] = np.asarray(q[b])[sel]
        xTs_sel.append(np.ascontiguousarray(xs.T).astype(bf))
        padvs.append(
            np.ascontiguousarray(
                (np.arange(NKT_C * 128) < n_sel)
                .astype(np.float32)
                .reshape(NKT_C, 128)
                .T
            )
        )
        km = np.zeros((NQT, MAXBD, 128, QT), dtype=np.float32)
        for qi in range(NQT):
            qpos = qi * QT + np.arange(QT)
            for j in range(MAXBD):
                kt = cfg["bk0s"][qi] + j
                if kt >= cfg["nk_cs"][qi]:
                    continue
                idx = kt * 128 + np.arange(128)
                valid = idx < n_sel
                pos = np.where(valid, sel[np.minimum(idx, n_sel - 1)], -1)
                km[qi, j] = np.where(
                    valid[:, None] & (pos[:, None] > qpos[None, :]),
                    np.float32(NEG),
                    np.float32(0),
                )
        kmasks.append(np.ascontiguousarray(km.astype(bf)))
    for core in range(8):
        b, r = divmod(core, GROUP)
        sl = slice(r * HC, (r + 1) * HC)
        in_maps.append(
            {
                "xT": xTs_full[b],
                "xTs": xTs_sel[b],
                "wqT": np.ascontiguousarray(Wq[sl, :].T).astype(bf),
                "wkT": np.ascontiguousarray(Wk[sl, :].T).astype(bf),
                "wvT": np.ascontiguousarray(Wv[sl, :].T).astype(bf),
                "woT": np.ascontiguousarray(Wo[sl, :].T).astype(bf),
                "bq": np.ascontiguousarray(bq[sl].reshape(2, 128).T).astype(np.float32),
                "bk": np.ascontiguousarray(bk[sl].reshape(2, 128).T).astype(np.float32),
                "bv": np.ascontiguousarray(bv[sl].reshape(1, HC)).astype(bf),
                "bo": np.ascontiguousarray(bo[sl].reshape(2, 128).T).astype(np.float32),
                "padv": padvs[b],
                "kmask": kmasks[b],
                "esel": esel_m,
            }
        )
    return in_maps


def assemble_output(results):
    full = np.empty((B, S, D), dtype=np.float32)
    for core in range(8):
        b, r = divmod(core, GROUP)
        full[b, :, r * HC : (r + 1) * HC] = results[core]["outT"].T
    return full


_NC_CACHE = [None]
_CFG_KEY = [None]


def kernel(**inputs):
    """Full-input MHA forward. inputs: q, pad_mask, Wq, bq, Wk, bk, Wv, bv,
    Wo, bo (as produced by setup_inputs). Returns [B, S, D] float32."""
    inputs = {k: np.asarray(v) for k, v in inputs.items()}
    mask_key = inputs["pad_mask"].tobytes()
    if _NC_CACHE[0] is None or _CFG_KEY[0] != mask_key:
        _NC_CACHE[0] = build(compact_cfg(inputs["pad_mask"]))
        _CFG_KEY[0] = mask_key
    nc = _NC_CACHE[0]
    in_maps = make_inputs(**inputs)
    res = run_bass_kernel_spmd(nc, in_maps, list(range(8)))
    return assemble_output(res.results)


# revision 27
# speedup vs baseline: 1.0860x; 1.0860x over previous
"""Multi-head attention (B=2, S=2048, D=1024, H=16, causal + key-pad mask)
as an 8-core Trainium2 Bass/Tile SPMD kernel.

Sharding: data parallel over the 2 batches (4 cores each); within a batch
group, tensor parallel over heads (4 heads/core) for the QKV projections and
attention. Head outputs are softmax-normalized on the owning core, cast to
bf16 and AllGathered per 512-wide q-tile; the O-projection is column-sliced
(each core computes its own 256 output columns for ALL rows) and pipelined
per q-tile into the attention stream, so only the last AllGather plus one
small O-chunk sits on the serial tail.

All matmul operands are bf16 (fp32 PSUM accumulation): bf16 streams at
1 cyc/row on the PE where f32r measured ~2.7, and it halves HBM/SBUF/wire
traffic. The key-pad mask is folded into the V projection (masked key rows
and their denominator 'ones' column are zeroed), so exp needs no bias.
Causal masking: block-level loop bounds + a -1e9 triangular DVE add on
diagonal blocks. Warm-up and keep-alive matmuls prevent the PE HAM clock
from dropping to half rate during DMA/collective-only windows.

self-contained: includes a workaround for the walrus per-instruction
sync-wait limit and an NTFF-profile hook shim.
"""
import sys
import types

import numpy as np
import ml_dtypes

import bass_rust
import concourse.bass as bass
import concourse.mybir as mybir
import concourse.tile as tile


# ---- walrus sync-wait limit workaround ----------------------------------
# This walrus build rejects instructions carrying more than one sem wait
# ("Too many sync wait commands"). Tile emits multi-wait instructions (the
# final drain, matmuls waiting on several DMA queues). Split excess waits
# onto same-engine NoOps placed immediately before the instruction --
# serial waits on one sequencer are semantically identical.
_WSPLIT_COUNTER = [0]


def _split_excess_waits(nc, limit=1):
    for fn in nc.m.functions:
        for bb in fn.blocks:
            out = []
            changed = False
            for inst in bb.instructions:
                si = inst.sync_info
                waits = list(si.on_wait) if si is not None and si.on_wait else []
                if len(waits) > limit:
                    extra, keep = waits[:-limit], waits[-limit:]
                    for s in range(0, len(extra), limit):
                        _WSPLIT_COUNTER[0] += 1
                        nop = mybir.InstNoOp(
                            name=f"I-wsplit-{_WSPLIT_COUNTER[0]}", ins=[], outs=[]
                        )
                        nop.engine = inst.engine
                        nop.sync_info = bass_rust.SyncInfo(
                            on_wait=extra[s : s + limit], on_update=[]
                        )
                        out.append(nop)
                    si.on_wait = keep
                    changed = True
                out.append(inst)
            if changed:
                bb.instructions = out


def _install_tile_patch():
    if getattr(tile.TileContext, "_wait_split_patched", False):
        return
    orig_exit = tile.TileContext.__exit__

    def __exit__(self, exc_type, exc_val, exc_tb):
        r = orig_exit(self, exc_type, exc_val, exc_tb)
        if exc_type is None:
            _split_excess_waits(self.nc)
        return r

    tile.TileContext.__exit__ = __exit__
    tile.TileContext._wait_split_patched = True


_install_tile_patch()


# ---- NTFF profile hook shim (axon deployments missing antenv.axon_hooks) --
def _install_ntff_hook():
    try:
        import antenv.axon_hooks  # noqa: F401
        return
    except ImportError:
        pass
    try:
        from trn_agent_boot.trn_boot import _ntff_profile_via_ctypes

        hook = _ntff_profile_via_ctypes("/opt/axon/libaxon_pjrt.so")
    except Exception:
        hook = None
    m = types.ModuleType("antenv.axon_hooks")
    m.get_axon_ntff_profile_hook = lambda: hook
    m.set_axon_ntff_profile_hook = lambda h: None
    sys.modules["antenv.axon_hooks"] = m


_install_ntff_hook()

from concourse.bass_utils import run_bass_kernel_spmd  # noqa: E402

f32 = mybir.dt.float32
f32r = mybir.dt.float32r
bf16 = mybir.dt.bfloat16

B, S, D, H, HD = 2, 2048, 1024, 16, 64
HPC, GROUP = 4, 4          # heads per core, cores per batch
HC = HPC * HD              # 256 projection cols per core
NKT = S // 128             # 16 k-tiles
NQT = S // 512             # 4 q-tiles
QT = 512                   # q-tile width
SCALE = 1.0 / np.sqrt(HD)  # 0.125
NEG = -1.0e9
KCH = D // 128             # 8 contraction chunks

REPLICA_GROUPS = [[0, 1, 2, 3], [4, 5, 6, 7]]

ADD = mybir.AluOpType.add
MULT = mybir.AluOpType.mult
EXP = mybir.ActivationFunctionType.Exp


def r32(ap):
    return ap.bitcast(f32r)


def build(cfg):
    # cfg: dict with NKT_C (compacted k-tiles), NCK (k-proj 512-col units),
    # nk_cs[NQT], bk0s[NQT], MAXBD (mask tiles per q-tile)
    nc = bass.Bass()
    dp = nc.declare_dram_parameter
    SK = cfg["NCK"] * QT
    xT = dp("xT", [D, S], bf16, isOutput=False)
    xTs = dp("xTs", [D, SK], bf16, isOutput=False)     # compacted keys (padded)
    wqT = dp("wqT", [D, HC], bf16, isOutput=False)
    wkT = dp("wkT", [D, HC], bf16, isOutput=False)
    wvT = dp("wvT", [D, HC], bf16, isOutput=False)
    woT = dp("woT", [D, HC], bf16, isOutput=False)     # this core's 256 out cols
    bq = dp("bq", [128, 2], f32, isOutput=False)
    bk = dp("bk", [128, 2], f32, isOutput=False)
    bv = dp("bv", [1, HC], bf16, isOutput=False)
    bo = dp("bo", [128, 2], f32, isOutput=False)       # this core's 256 out cols
    padv = dp("padv", [128, cfg["NKT_C"]], f32, isOutput=False)  # 1 real / 0 pad
    kmask = dp("kmask", [NQT, cfg["MAXBD"], 128, QT], bf16, isOutput=False)
    esel = dp("esel", [H, D], bf16, isOutput=False)     # head-of-column selector
    outT = dp("outT", [HC, S], f32, isOutput=True)

    with tile.TileContext(nc) as tc:
        _body(nc, tc, cfg, locals())
    return nc


def _body(nc, tc, cfg, t):
    xT, wqT, wkT, wvT, woT = t["xT"], t["wqT"], t["wkT"], t["wvT"], t["woT"]
    bq, bk, bv, bo, padv = t["bq"], t["bk"], t["bv"], t["bo"], t["padv"]
    xTs, kmask = t["xTs"], t["kmask"]
    esel = t["esel"]
    outT = t["outT"]
    NKT_C, NCK = cfg["NKT_C"], cfg["NCK"]
    nk_cs, bk0s, MAXBD = cfg["nk_cs"], cfg["bk0s"], cfg["MAXBD"]
    SK = NCK * QT

    ctx_pools = []

    def pool(name, bufs, space="SBUF"):
        p = tc.tile_pool(name=name, bufs=bufs, space=space)
        ctx_pools.append(p)
        return p.__enter__()

    dram_pool = pool("dram", 1, space="DRAM")
    HR = HD + 1  # 65 rows/head: 64 o-rows + softmax denominator
    ag_in = dram_pool.tile([NQT, HPC * HR, QT], bf16)       # [4, 260, 512]
    ag_out = dram_pool.tile([NQT, H * HR, QT], bf16)        # [4, 1040, 512]
    agw_in = dram_pool.tile([64, 64], bf16)
    agw_out = dram_pool.tile([GROUP * 64, 64], bf16)

    const = pool("const", 1)
    probs_pool = pool("probs", 2)
    agsb_pool = pool("agsb", 2)
    stg_pool = pool("stg", 3)
    outsb_pool = pool("outsb", 2)

    ps_st = pool("ps_st", 2, space="PSUM")
    ps_ot = pool("ps_ot", 2, space="PSUM")
    ps_mm = pool("ps_mm", 2, space="PSUM")

    # ---- PE warm-up: ~4us of dependency-free matmuls so the HAM clock ----
    # gate opens while the input DMAs stream in.
    wu_a = const.tile([128, 128], bf16)
    nc.any.memset(wu_a[:], 0.015625)
    wu_b = const.tile([128, QT], bf16)
    nc.any.memset(wu_b[:], 0.015625)

    def warm_mms(n):
        for _ in range(n):
            ps = ps_mm.tile([128, QT], f32, tag="mm")
            nc.tensor.matmul(ps[:], wu_a[:], wu_b[:], start=True, stop=True)

    warm_mms(40)

    # ---- resident inputs -------------------------------------------------
    xt = const.tile([128, KCH, S], bf16)      # xT, chunk-major (queries)
    xts = const.tile([128, KCH, SK], bf16)    # compacted keys, chunk-major
    wq_t = const.tile([128, KCH, HC], bf16)
    wk_t = const.tile([128, KCH, HC], bf16)
    wv_t = const.tile([128, KCH, HC], bf16)
    wo_t = const.tile([128, KCH, HC], bf16)
    bq_t = const.tile([128, 2], f32)
    nc.sync.dma_start(bq_t[:], bq[:])
    bk_t = const.tile([128, 2], f32)
    nc.sync.dma_start(bk_t[:], bk[:])
    bo_t = const.tile([128, 2], f32)
    nc.sync.dma_start(bo_t[:], bo[:])
    padv_t = const.tile([128, NKT_C], f32)
    nc.sync.dma_start(padv_t[:], padv[:])
    km_t = const.tile([128, NQT, MAXBD, QT], bf16)
    for n in range(NQT):
        for m in range(MAXBD):
            nc.scalar.dma_start(km_t[:, n, m, :], kmask[n, m])
    bv_row = const.tile([128, HC], bf16)
    nc.any.memset(bv_row[:], 0.0)
    nc.sync.dma_start(bv_row[0:1, :], bv[:])

    wqT_r = wqT.rearrange("(c p) j -> p c j", p=128)
    wkT_r = wkT.rearrange("(c p) j -> p c j", p=128)
    wvT_r = wvT.rearrange("(c p) j -> p c j", p=128)
    woT_r = woT.rearrange("(c p) j -> p c j", p=128)
    xT_r = xT.rearrange("(c p) s -> p c s", p=128)
    xTs_r = xTs.rearrange("(c p) s -> p c s", p=128)
    # q-tile-0 critical path on the sync queue: whole weight tensors, then
    # per-chunk x loads so the first projections pipeline
    nc.sync.dma_start(wk_t[:], wkT_r[:])
    nc.sync.dma_start(wq_t[:], wqT_r[:])
    nc.sync.dma_start(wv_t[:], wvT_r[:])
    for k in range(KCH):
        nc.sync.dma_start(xts[:, k, 0:QT], xTs_r[:, k, 0:QT])
        nc.sync.dma_start(xt[:, k, 0:QT], xT_r[:, k, 0:QT])
    # bulk x loads ride the gpsimd queue (compute-free until the first real
    # AllGather), then the warm-up AllGather absorbs the TOPSP cold start
    for k in range(KCH):
        nc.gpsimd.dma_start(xts[:, k, QT:SK], xTs_r[:, k, QT:SK])
    for k in range(KCH):
        nc.gpsimd.dma_start(xt[:, k, QT:S], xT_r[:, k, QT:S])
    nc.gpsimd.dma_start(agw_in[:], wu_a[0:64, 0:64])
    nc.gpsimd.collective_compute(
        "AllGather",
        mybir.AluOpType.bypass,
        replica_groups=REPLICA_GROUPS,
        ins=[agw_in[:]],
        outs=[agw_out[:]],
    )
    for k in range(KCH):
        nc.gpsimd.dma_start(wo_t[:, k], woT_r[:, k])

    # e0: row 0 = ones, rest 0 -- broadcast-matmul stationary
    e0h = const.tile([128, 128], bf16)
    nc.any.memset(e0h[:], 0.0)
    ones_row_h = const.tile([1, 128], bf16)
    nc.any.memset(ones_row_h[:], 1.0)
    nc.vector.tensor_copy(e0h[0:1, :], ones_row_h[0:1, :])

    # bvb: bv broadcast to all 128 partitions (e0h row 0 is ones, rest 0)
    bvb = const.tile([128, HC], f32)
    bv_ps = ps_mm.tile([128, HC], f32, tag="mm")
    nc.tensor.matmul(bv_ps[:], e0h[:], bv_row[:], start=True, stop=True)
    nc.vector.tensor_copy(bvb[:], bv_ps[:])

    # esel stationary (rows 16-127 zero) and reciprocal-broadcast tiles
    esel_t = const.tile([128, KCH, 128], bf16)
    nc.any.memset(esel_t[:], 0.0)
    nc.sync.dma_start(
        esel_t[0:H], esel.rearrange("h (c n) -> h c n", n=128)
    )
    rec16_a = const.tile([128, QT], bf16)
    nc.any.memset(rec16_a[:], 0.0)
    rec16_b = const.tile([128, QT], bf16)
    nc.any.memset(rec16_b[:], 0.0)
    rec16s = (rec16_a, rec16_b)

    # projection outputs
    qh_t = const.tile([128, 2, S], bf16)    # [j-in-tile, j-tile(pair), s]
    kh_t = const.tile([128, 2, SK], bf16)
    vh_t = const.tile([128, NKT_C, HPC, HD + 1], bf16)
    # denominator 'ones' column: 1.0 for real keys, 0.0 for padding
    nc.vector.tensor_copy(
        vh_t[:, :, :, HD : HD + 1],
        padv_t[:].rearrange("p (t u v) -> p t u v", u=1, v=1).broadcast_to(
            [128, NKT_C, HPC, 1]
        ),
    )

    def proj_qk(w_t, b_t, out_t, jt, c, x_src):
        # one [128, 512] tile of qhT/khT: out partition=j, free=s
        ps = ps_mm.tile([128, QT], f32, tag="mm")
        for k in range(KCH):
            nc.tensor.matmul(
                ps[:],
                w_t[:, k, jt * 128 : (jt + 1) * 128],
                x_src[:, k, c * QT : (c + 1) * QT],
                start=(k == 0),
                stop=(k == KCH - 1),
            )
            if k % 2 == 1:
                yield
        nc.vector.tensor_scalar_add(
            out_t[:, jt, c * QT : (c + 1) * QT], ps[:], b_t[:, jt : jt + 1]
        )

    def proj_v(st_):
        """one s-tile of vh: out partition=s, free=[4 heads x 64]; masked rows zeroed."""
        ps = ps_mm.tile([128, HC], f32, tag="mm")
        for k in range(KCH):
            nc.tensor.matmul(
                ps[:],
                xts[:, k, st_ * 128 : (st_ + 1) * 128],
                wv_t[:, k, :],
                start=(k == 0),
                stop=(k == KCH - 1),
            )
            if k % 2 == 1:
                yield
        vsl = vh_t[:, st_, :, 0:HD]
        nc.vector.tensor_tensor(
            vsl,
            ps[:].rearrange("p (h d) -> p h d", h=HPC),
            bvb[:].rearrange("p (h d) -> p h d", h=HPC),
            ADD,
        )
        nc.vector.tensor_scalar_mul(vsl, vsl, padv_t[:, st_ : st_ + 1])

    agsb_tiles = {}
    den_tiles = {}

    def norm_recip(qi):
        # one batched reciprocal of all 16 gathered denominators
        den_sb = den_tiles[qi]
        den_f = stg_pool.tile([H, QT], f32, tag="denf")
        nc.vector.tensor_copy(den_f[:], den_sb[:])
        rec_f = stg_pool.tile([H, QT], f32, tag="recf")
        nc.vector.reciprocal(rec_f[:], den_f[:])
        rec16 = rec16s[qi % 2]
        with nc.allow_low_precision(reason="bf16 staging precision"):
            nc.vector.tensor_copy(rec16[0:H, :], rec_f[:])
        yield

    def norm_chunks(qi, cs):
        # broadcast 1/den per head over its 64 rows; scale agsb in place
        agsb = agsb_tiles[qi]
        rec16 = rec16s[qi % 2]
        for c in cs:
            bcst_ps = ps_mm.tile([128, QT], f32, tag="mm")
            nc.tensor.matmul(
                bcst_ps[:], esel_t[:, c, :], rec16[:], start=True, stop=True
            )
            with nc.allow_low_precision(reason="bf16 staging precision"):
                nc.vector.tensor_tensor(
                    agsb[:, c, :], agsb[:, c, :], bcst_ps[:], MULT
                )
            yield

    def o_chunk(qi, jh):
        """O-projection for q-tile qi, output-column half jh (128 j cols)."""
        agsb = agsb_tiles[qi]
        ps = ps_mm.tile([128, QT], f32, tag="mm")
        for dc in range(KCH):
            nc.tensor.matmul(
                ps[:],
                wo_t[:, dc, jh * 128 : (jh + 1) * 128],
                agsb[:, dc, :],
                start=(dc == 0),
                stop=(dc == KCH - 1),
            )
            if dc % 2 == 1:
                yield
        osb = outsb_pool.tile([128, QT], f32, tag="osb")
        nc.vector.tensor_scalar_add(osb[:], ps[:], bo_t[:, jh : jh + 1])
        nc.sync.dma_start(
            outT[jh * 128 : (jh + 1) * 128, qi * QT : (qi + 1) * QT], osb[:]
        )

    def attention_qtile(qi, filler):
        q0 = qi * QT
        nk = nk_cs[qi]
        bk0 = bk0s[qi]
        for pair in range(2):
            probs = probs_pool.tile([128, NKT_C, 2, QT], bf16, tag="probs")

            def emit_sc(kt):
                k0 = kt * 128
                st = ps_st.tile([128, 2, QT], f32, tag="st")
                for hh in range(2):
                    nc.tensor.matmul(
                        st[:, hh, :],
                        kh_t[hh * 64 : hh * 64 + 64, pair, k0 : k0 + 128],
                        qh_t[hh * 64 : hh * 64 + 64, pair, q0 : q0 + QT],
                        start=True,
                        stop=True,
                    )
                if kt >= bk0:  # causal staircase mask (host-precomputed)
                    nc.vector.tensor_tensor(
                        st[:, :, :],
                        st[:, :, :],
                        km_t[:, qi, kt - bk0, :].rearrange(
                            "p (o n) -> p o n", o=1
                        ).broadcast_to([128, 2, QT]),
                        ADD,
                    )
                nc.scalar.activation(
                    probs[:, kt, :, :],
                    st[:, :, :],
                    EXP,
                    bias=0.0,
                    scale=float(SCALE),
                )

            emit_sc(0)
            ot0 = ps_ot.tile([HD + 1, QT], f32, tag="ot")
            ot1 = ps_ot.tile([HD + 1, QT], f32, tag="ot")
            ots = (ot0, ot1)
            for kt in range(nk):
                if kt + 1 < nk:
                    emit_sc(kt + 1)
                for hh in range(2):
                    h = 2 * pair + hh
                    nc.tensor.matmul(
                        ots[hh][:, :],
                        vh_t[:, kt, h, :],
                        probs[:, kt, hh, :],
                        start=(kt == 0),
                        stop=(kt == nk - 1),
                    )
                filler()
            # stage unnormalized o (rows 0-255) + denominator (rows 256-259)
            for hh in range(2):
                h = 2 * pair + hh
                stg = stg_pool.tile([HD + 1, QT], bf16, tag="stg")
                nc.vector.tensor_copy(stg[:], ots[hh][:])
                nc.sync.dma_start(
                    ag_in[qi, h * HD : (h + 1) * HD, :], stg[0:HD, :]
                )
                nc.sync.dma_start(
                    ag_in[qi, HPC * HD + h : HPC * HD + h + 1, :], stg[HD : HD + 1, :]
                )

    # ---- emission: projections + O-chunks finely interleaved -------------
    def kneed(qi):
        return min(NCK, (nk_cs[qi] * 128 + QT - 1) // QT)

    emitted = {"kc": 0, "qc": 0, "vst": 0}

    def proj_units_for(qi):
        # units that must complete before attention(qi) runs
        units = []
        for c in range(emitted["kc"], kneed(qi)):
            for jt in range(2):
                units.append(
                    lambda jt=jt, c=c: proj_qk(wk_t, bk_t, kh_t, jt, c, xts)
                )
        emitted["kc"] = max(emitted["kc"], kneed(qi))
        for c in range(emitted["qc"], qi + 1):
            for jt in range(2):
                units.append(
                    lambda jt=jt, c=c: proj_qk(wq_t, bq_t, qh_t, jt, c, xt)
                )
        emitted["qc"] = max(emitted["qc"], qi + 1)
        for st_ in range(emitted["vst"], nk_cs[qi]):
            units.append(lambda st_=st_: proj_v(st_))
        emitted["vst"] = max(emitted["vst"], nk_cs[qi])
        return units

    def o_chunk_units(qi):
        return (
            [lambda qi=qi: norm_recip(qi)]
            + [
                lambda qi=qi, cs=cs: norm_chunks(qi, cs)
                for cs in ((0, 1, 2, 3), (4, 5, 6, 7))
            ]
            + [lambda jh=jh, qi=qi: o_chunk(qi, jh) for jh in range(2)]
        )

    class Filler:
        def __init__(self, units, budget):
            self.units = list(units)
            self.gen = None
            self.budget = budget

        def __call__(self):
            for _ in range(self.budget):
                if self.gen is None:
                    if not self.units:
                        return
                    self.gen = self.units.pop(0)()
                try:
                    next(self.gen)
                except StopIteration:
                    self.gen = None

        def flush(self):
            while self.units or self.gen is not None:
                if self.gen is None:
                    self.gen = self.units.pop(0)()
                for _ in self.gen:
                    pass
                self.gen = None

    Filler(proj_units_for(0), 1).flush()
    for qi in range(NQT):
        pending = proj_units_for(qi + 1) if qi + 1 < NQT else []
        n_att = 2 * nk_cs[qi]             # filler() call sites this q-tile
        total_steps = len(pending) * 5
        budget = max(1, (total_steps + n_att - 1) // n_att)
        filler = Filler(pending, budget)
        attention_qtile(qi, filler)
        filler.flush()
        # AllGather this q-tile's head outputs across the group
        nc.gpsimd.collective_compute(
            "AllGather",
            mybir.AluOpType.bypass,
            replica_groups=REPLICA_GROUPS,
            ins=[ag_in[qi]],
            outs=[ag_out[qi]],
        )
        agsb = agsb_pool.tile([128, KCH, QT], bf16, tag="agsb")
        agsb_tiles[qi] = agsb
        den_sb = agsb_pool.tile([H, QT], bf16, tag="den")
        den_tiles[qi] = den_sb
        RB = HPC * HR  # 260 rows per rank block
        for r in range(GROUP):
            nc.gpsimd.dma_start(
                agsb[:, 2 * r : 2 * r + 2, :],
                ag_out[qi, r * RB : r * RB + HPC * HD, :].rearrange(
                    "(c p) q -> p c q", p=128
                ),
            )
            nc.gpsimd.dma_start(
                den_sb[HPC * r : HPC * r + HPC, :],
                ag_out[qi, r * RB + HPC * HD : (r + 1) * RB, :],
            )
        # normalize + O-project q-tile qi-2 (its AllGather finished two
        # attention spans ago) while AG(qi) is on the wire
        if qi >= 2:
            Filler(o_chunk_units(qi - 2), 1).flush()

    # tail: O-chunk for q-tile 2, keep the PE clock warm while the last
    # AllGather drains, then the final O-chunk
    Filler(o_chunk_units(NQT - 2), 1).flush()
    warm_mms(110)
    Filler(o_chunk_units(NQT - 1), 1).flush()

    for p in reversed(ctx_pools):
        p.__exit__(None, None, None)


# ---- host-side marshalling ----------------------------------------------


def compact_cfg(pad_mask):
    """Key-compaction geometry shared by both batches (max-padded)."""
    pad_mask = np.asarray(pad_mask)
    sels = [np.where(~pad_mask[b])[0] for b in range(B)]

    def cnt(b, p):
        return int(np.searchsorted(sels[b], p))

    nk_cs, bk0s = [], []
    for qi in range(NQT):
        q0 = qi * QT
        c_end = max(cnt(b, q0 + QT) for b in range(B))
        nk_cs.append(max(1, -(-c_end // 128)))
        bk0s.append(min(cnt(b, q0) // 128 for b in range(B)))
    return {
        "NKT_C": nk_cs[-1],
        "NCK": -(-(nk_cs[-1] * 128) // QT),
        "nk_cs": nk_cs,
        "bk0s": bk0s,
        "MAXBD": max(nk_cs[i] - bk0s[i] for i in range(NQT)),
        "sels": sels,
    }


def make_inputs(q, pad_mask, Wq, bq, Wk, bk, Wv, bv, Wo, bo):
    """Build the 8 per-core input maps from full inputs."""
    bf = ml_dtypes.bfloat16
    cfg = compact_cfg(pad_mask)
    NKT_C, NCK, MAXBD = cfg["NKT_C"], cfg["NCK"], cfg["MAXBD"]
    SK = NCK * QT
    esel_m = np.ascontiguousarray(
        (np.arange(D)[None, :] // HD == np.arange(H)[:, None]).astype(bf)
    )
    in_maps = []
    xTs_full = [np.ascontiguousarray(q[b].T).astype(bf) for b in range(B)]
    xTs_sel, padvs, kmasks = [], [], []
    for b in range(B):
        sel = cfg["sels"][b]
        n_sel = len(sel)
        xs = np.zeros((SK, D), dtype=np.float32)
        xs[:n_sel] = np.asarray(q[b])[sel]
        xTs_sel.append(np.ascontiguousarray(xs.T).astype(bf))
        padvs.append(
            np.ascontiguousarray(
                (np.arange(NKT_C * 128) < n_sel)
                .astype(np.float32)
                .reshape(NKT_C, 128)
                .T
            )
        )
        km = np.zeros((NQT, MAXBD, 128, QT), dtype=np.float32)
        for qi in range(NQT):
            qpos = qi * QT + np.arange(QT)
            for j in range(MAXBD):
                kt = cfg["bk0s"][qi] + j
                if kt >= cfg["nk_cs"][qi]:
                    continue
                idx = kt * 128 + np.arange(128)
                valid = idx < n_sel
                pos = np.where(valid, sel[np.minimum(idx, n_sel - 1)], -1)
                km[qi, j] = np.where(
                    valid[:, None] & (pos[:, None] > qpos[None, :]),
                    np.float32(NEG),
                    np.float32(0),
                )
        kmasks.append(np.ascontiguousarray(km.astype(bf)))
    for core in range(8):
        b, r = divmod(core, GROUP)
        sl = slice(r * HC, (r + 1) * HC)
        in_maps.append(
            {
                "xT": xTs_full[b],
                "xTs": xTs_sel[b],
                "wqT": np.ascontiguousarray(Wq[sl, :].T).astype(bf),
                "wkT": np.ascontiguousarray(Wk[sl, :].T).astype(bf),
                "wvT": np.ascontiguousarray(Wv[sl, :].T).astype(bf),
                "woT": np.ascontiguousarray(Wo[sl, :].T).astype(bf),
                "bq": np.ascontiguousarray(bq[sl].reshape(2, 128).T).astype(np.float32),
                "bk": np.ascontiguousarray(bk[sl].reshape(2, 128).T).astype(np.float32),
                "bv": np.ascontiguousarray(bv[sl].reshape(1, HC)).astype(bf),
                "bo": np.ascontiguousarray(bo[sl].reshape(2, 128).T).astype(np.float32),
                "padv": padvs[b],
                "kmask": kmasks[b],
                "esel": esel_m,
            }
        )
    return in_maps


def assemble_output(results):
    full = np.empty((B, S, D), dtype=np.float32)
    for core in range(8):
        b, r = divmod(core, GROUP)
        full[b, :, r * HC : (r + 1) * HC] = results[core]["outT"].T
    return full


_NC_CACHE = [None]
_CFG_KEY = [None]


def kernel(**inputs):
    """Full-input MHA forward. inputs: q, pad_mask, Wq, bq, Wk, bk, Wv, bv,
    Wo, bo (as produced by setup_inputs). Returns [B, S, D] float32."""
    inputs = {k: np.asarray(v) for k, v in inputs.items()}
    mask_key = inputs["pad_mask"].tobytes()
    if _NC_CACHE[0] is None or _CFG_KEY[0] != mask_key:
        _NC_CACHE[0] = build(compact_cfg(inputs["pad_mask"]))
        _CFG_KEY[0] = mask_key
    nc = _NC_CACHE[0]
    in_maps = make_inputs(**inputs)
    res = run_bass_kernel_spmd(nc, in_maps, list(range(8)))
    return assemble_output(res.results)


# revision 29
# speedup vs baseline: 1.0886x; 1.0024x over previous
"""Multi-head attention (B=2, S=2048, D=1024, H=16, causal + key-pad mask)
as an 8-core Trainium2 Bass/Tile SPMD kernel.

Sharding: data parallel over the 2 batches (4 cores each); within a batch
group, tensor parallel over heads (4 heads/core) for the QKV projections and
attention. Head outputs are softmax-normalized on the owning core, cast to
bf16 and AllGathered per 512-wide q-tile; the O-projection is column-sliced
(each core computes its own 256 output columns for ALL rows) and pipelined
per q-tile into the attention stream, so only the last AllGather plus one
small O-chunk sits on the serial tail.

All matmul operands are bf16 (fp32 PSUM accumulation): bf16 streams at
1 cyc/row on the PE where f32r measured ~2.7, and it halves HBM/SBUF/wire
traffic. The key-pad mask is folded into the V projection (masked key rows
and their denominator 'ones' column are zeroed), so exp needs no bias.
Causal masking: block-level loop bounds + a -1e9 triangular DVE add on
diagonal blocks. Warm-up and keep-alive matmuls prevent the PE HAM clock
from dropping to half rate during DMA/collective-only windows.

self-contained: includes a workaround for the walrus per-instruction
sync-wait limit and an NTFF-profile hook shim.
"""
import sys
import types

import numpy as np
import ml_dtypes

import bass_rust
import concourse.bass as bass
import concourse.mybir as mybir
import concourse.tile as tile


# ---- walrus sync-wait limit workaround ----------------------------------
# This walrus build rejects instructions carrying more than one sem wait
# ("Too many sync wait commands"). Tile emits multi-wait instructions (the
# final drain, matmuls waiting on several DMA queues). Split excess waits
# onto same-engine NoOps placed immediately before the instruction --
# serial waits on one sequencer are semantically identical.
_WSPLIT_COUNTER = [0]


def _split_excess_waits(nc, limit=1):
    for fn in nc.m.functions:
        for bb in fn.blocks:
            out = []
            changed = False
            for inst in bb.instructions:
                si = inst.sync_info
                waits = list(si.on_wait) if si is not None and si.on_wait else []
                if len(waits) > limit:
                    extra, keep = waits[:-limit], waits[-limit:]
                    for s in range(0, len(extra), limit):
                        _WSPLIT_COUNTER[0] += 1
                        nop = mybir.InstNoOp(
                            name=f"I-wsplit-{_WSPLIT_COUNTER[0]}", ins=[], outs=[]
                        )
                        nop.engine = inst.engine
                        nop.sync_info = bass_rust.SyncInfo(
                            on_wait=extra[s : s + limit], on_update=[]
                        )
                        out.append(nop)
                    si.on_wait = keep
                    changed = True
                out.append(inst)
            if changed:
                bb.instructions = out


def _install_tile_patch():
    if getattr(tile.TileContext, "_wait_split_patched", False):
        return
    orig_exit = tile.TileContext.__exit__

    def __exit__(self, exc_type, exc_val, exc_tb):
        r = orig_exit(self, exc_type, exc_val, exc_tb)
        if exc_type is None:
            _split_excess_waits(self.nc)
        return r

    tile.TileContext.__exit__ = __exit__
    tile.TileContext._wait_split_patched = True


_install_tile_patch()


# ---- NTFF profile hook shim (axon deployments missing antenv.axon_hooks) --
def _install_ntff_hook():
    try:
        import antenv.axon_hooks  # noqa: F401
        return
    except ImportError:
        pass
    try:
        from trn_agent_boot.trn_boot import _ntff_profile_via_ctypes

        hook = _ntff_profile_via_ctypes("/opt/axon/libaxon_pjrt.so")
    except Exception:
        hook = None
    m = types.ModuleType("antenv.axon_hooks")
    m.get_axon_ntff_profile_hook = lambda: hook
    m.set_axon_ntff_profile_hook = lambda h: None
    sys.modules["antenv.axon_hooks"] = m


_install_ntff_hook()

from concourse.bass_utils import run_bass_kernel_spmd  # noqa: E402

f32 = mybir.dt.float32
f32r = mybir.dt.float32r
bf16 = mybir.dt.bfloat16

B, S, D, H, HD = 2, 2048, 1024, 16, 64
HPC, GROUP = 4, 4          # heads per core, cores per batch
HC = HPC * HD              # 256 projection cols per core
NKT = S // 128             # 16 k-tiles
NQT = S // 512             # 4 q-tiles
QT = 512                   # q-tile width
SCALE = 1.0 / np.sqrt(HD)  # 0.125
NEG = -1.0e9
KCH = D // 128             # 8 contraction chunks

REPLICA_GROUPS = [[0, 1, 2, 3], [4, 5, 6, 7]]

ADD = mybir.AluOpType.add
MULT = mybir.AluOpType.mult
EXP = mybir.ActivationFunctionType.Exp


def r32(ap):
    return ap.bitcast(f32r)


def build(cfg):
    # cfg: dict with NKT_C (compacted k-tiles), NCK (k-proj 512-col units),
    # nk_cs[NQT], bk0s[NQT], MAXBD (mask tiles per q-tile)
    nc = bass.Bass()
    dp = nc.declare_dram_parameter
    SK = cfg["NCK"] * QT
    xT = dp("xT", [D, S], bf16, isOutput=False)
    xTs = dp("xTs", [D, SK], bf16, isOutput=False)     # compacted keys (padded)
    wqT = dp("wqT", [D, HC], bf16, isOutput=False)
    wkT = dp("wkT", [D, HC], bf16, isOutput=False)
    wvT = dp("wvT", [D, HC], bf16, isOutput=False)
    woT = dp("woT", [D, HC], bf16, isOutput=False)     # this core's 256 out cols
    bq = dp("bq", [128, 2], f32, isOutput=False)
    bk = dp("bk", [128, 2], f32, isOutput=False)
    bv = dp("bv", [1, HC], bf16, isOutput=False)
    bo = dp("bo", [128, 2], f32, isOutput=False)       # this core's 256 out cols
    padv = dp("padv", [128, cfg["NKT_C"]], f32, isOutput=False)  # 1 real / 0 pad
    kmask = dp("kmask", [NQT, cfg["MAXBD"], 128, QT], bf16, isOutput=False)
    esel = dp("esel", [H, D], bf16, isOutput=False)     # head-of-column selector
    outT = dp("outT", [HC, S], f32, isOutput=True)

    with tile.TileContext(nc) as tc:
        _body(nc, tc, cfg, locals())
    return nc


def _body(nc, tc, cfg, t):
    xT, wqT, wkT, wvT, woT = t["xT"], t["wqT"], t["wkT"], t["wvT"], t["woT"]
    bq, bk, bv, bo, padv = t["bq"], t["bk"], t["bv"], t["bo"], t["padv"]
    xTs, kmask = t["xTs"], t["kmask"]
    esel = t["esel"]
    outT = t["outT"]
    NKT_C, NCK = cfg["NKT_C"], cfg["NCK"]
    nk_cs, bk0s, MAXBD = cfg["nk_cs"], cfg["bk0s"], cfg["MAXBD"]
    SK = NCK * QT

    ctx_pools = []

    def pool(name, bufs, space="SBUF"):
        p = tc.tile_pool(name=name, bufs=bufs, space=space)
        ctx_pools.append(p)
        return p.__enter__()

    dram_pool = pool("dram", 1, space="DRAM")
    HR = HD + 1  # 65 rows/head: 64 o-rows + softmax denominator
    ag_in = dram_pool.tile([NQT, HPC * HR, QT], bf16)       # [4, 260, 512]
    ag_out = dram_pool.tile([NQT, H * HR, QT], bf16)        # [4, 1040, 512]
    agw_in = dram_pool.tile([64, 64], bf16)
    agw_out = dram_pool.tile([GROUP * 64, 64], bf16)

    const = pool("const", 1)
    probs_pool = pool("probs", 2)
    agsb_pool = pool("agsb", 2)
    stg_pool = pool("stg", 3)
    outsb_pool = pool("outsb", 2)

    ps_st = pool("ps_st", 2, space="PSUM")
    ps_ot = pool("ps_ot", 2, space="PSUM")
    ps_mm = pool("ps_mm", 2, space="PSUM")

    # ---- PE warm-up: ~4us of dependency-free matmuls so the HAM clock ----
    # gate opens while the input DMAs stream in.
    wu_a = const.tile([128, 128], bf16)
    nc.any.memset(wu_a[:], 0.015625)
    wu_b = const.tile([128, QT], bf16)
    nc.any.memset(wu_b[:], 0.015625)

    def warm_mms(n):
        for _ in range(n):
            ps = ps_mm.tile([128, QT], f32, tag="mm")
            nc.tensor.matmul(ps[:], wu_a[:], wu_b[:], start=True, stop=True)

    warm_mms(40)

    # ---- resident inputs -------------------------------------------------
    xt = const.tile([128, KCH, S], bf16)      # xT, chunk-major (queries)
    xts = const.tile([128, KCH, SK], bf16)    # compacted keys, chunk-major
    wq_t = const.tile([128, KCH, HC], bf16)
    wk_t = const.tile([128, KCH, HC], bf16)
    wv_t = const.tile([128, KCH, HC], bf16)
    wo_t = const.tile([128, KCH, HC], bf16)
    bq_t = const.tile([128, 2], f32)
    nc.sync.dma_start(bq_t[:], bq[:])
    bk_t = const.tile([128, 2], f32)
    nc.sync.dma_start(bk_t[:], bk[:])
    bo_t = const.tile([128, 2], f32)
    nc.sync.dma_start(bo_t[:], bo[:])
    padv_t = const.tile([128, NKT_C], f32)
    nc.sync.dma_start(padv_t[:], padv[:])
    km_t = const.tile([128, NQT, MAXBD, QT], bf16)
    for n in range(NQT):
        for m in range(MAXBD):
            nc.scalar.dma_start(km_t[:, n, m, :], kmask[n, m])
    bv_row = const.tile([128, HC], bf16)
    nc.any.memset(bv_row[:], 0.0)
    nc.sync.dma_start(bv_row[0:1, :], bv[:])

    wqT_r = wqT.rearrange("(c p) j -> p c j", p=128)
    wkT_r = wkT.rearrange("(c p) j -> p c j", p=128)
    wvT_r = wvT.rearrange("(c p) j -> p c j", p=128)
    woT_r = woT.rearrange("(c p) j -> p c j", p=128)
    xT_r = xT.rearrange("(c p) s -> p c s", p=128)
    xTs_r = xTs.rearrange("(c p) s -> p c s", p=128)
    # critical path (q-tile 0) on the sync queue; the rest on gpsimd's queue
    for k in range(KCH):
        nc.sync.dma_start(wk_t[:, k], wkT_r[:, k])
        nc.sync.dma_start(wq_t[:, k], wqT_r[:, k])
        nc.sync.dma_start(xt[:, k, 0:QT], xT_r[:, k, 0:QT])
        nc.sync.dma_start(xts[:, k, 0:QT], xTs_r[:, k, 0:QT])
        nc.sync.dma_start(wv_t[:, k], wvT_r[:, k])
    for c in range(1, NCK):
        for k in range(KCH):
            nc.gpsimd.dma_start(
                xts[:, k, c * QT : (c + 1) * QT], xTs_r[:, k, c * QT : (c + 1) * QT]
            )
    for c in range(1, NQT):
        for k in range(KCH):
            nc.gpsimd.dma_start(
                xt[:, k, c * QT : (c + 1) * QT], xT_r[:, k, c * QT : (c + 1) * QT]
            )
    # warm-up AllGather (after the bulk loads on this queue): absorbs the
    # TOPSP cold start so the first real AllGather runs at full rate
    nc.gpsimd.dma_start(agw_in[:], wu_a[0:64, 0:64])
    nc.gpsimd.collective_compute(
        "AllGather",
        mybir.AluOpType.bypass,
        replica_groups=REPLICA_GROUPS,
        ins=[agw_in[:]],
        outs=[agw_out[:]],
    )
    for k in range(KCH):
        nc.gpsimd.dma_start(wo_t[:, k], woT_r[:, k])

    # e0: row 0 = ones, rest 0 -- broadcast-matmul stationary
    e0h = const.tile([128, 128], bf16)
    nc.any.memset(e0h[:], 0.0)
    ones_row_h = const.tile([1, 128], bf16)
    nc.any.memset(ones_row_h[:], 1.0)
    nc.vector.tensor_copy(e0h[0:1, :], ones_row_h[0:1, :])

    # bvb: bv broadcast to all 128 partitions (e0h row 0 is ones, rest 0)
    bvb = const.tile([128, HC], f32)
    bv_ps = ps_mm.tile([128, HC], f32, tag="mm")
    nc.tensor.matmul(bv_ps[:], e0h[:], bv_row[:], start=True, stop=True)
    nc.vector.tensor_copy(bvb[:], bv_ps[:])

    # esel stationary (rows 16-127 zero) and reciprocal-broadcast tiles
    esel_t = const.tile([128, KCH, 128], bf16)
    nc.any.memset(esel_t[:], 0.0)
    nc.sync.dma_start(
        esel_t[0:H], esel.rearrange("h (c n) -> h c n", n=128)
    )
    rec16_a = const.tile([128, QT], bf16)
    nc.any.memset(rec16_a[:], 0.0)
    rec16_b = const.tile([128, QT], bf16)
    nc.any.memset(rec16_b[:], 0.0)
    rec16s = (rec16_a, rec16_b)

    # projection outputs
    qh_t = const.tile([128, 2, S], bf16)    # [j-in-tile, j-tile(pair), s]
    kh_t = const.tile([128, 2, SK], bf16)
    vh_t = const.tile([128, NKT_C, HPC, HD + 1], bf16)
    # denominator 'ones' column: 1.0 for real keys, 0.0 for padding
    nc.vector.tensor_copy(
        vh_t[:, :, :, HD : HD + 1],
        padv_t[:].rearrange("p (t u v) -> p t u v", u=1, v=1).broadcast_to(
            [128, NKT_C, HPC, 1]
        ),
    )

    def proj_qk(w_t, b_t, out_t, jt, c, x_src):
        # one [128, 512] tile of qhT/khT: out partition=j, free=s
        ps = ps_mm.tile([128, QT], f32, tag="mm")
        for k in range(KCH):
            nc.tensor.matmul(
                ps[:],
                w_t[:, k, jt * 128 : (jt + 1) * 128],
                x_src[:, k, c * QT : (c + 1) * QT],
                start=(k == 0),
                stop=(k == KCH - 1),
            )
            if k % 2 == 1:
                yield
        nc.vector.tensor_scalar_add(
            out_t[:, jt, c * QT : (c + 1) * QT], ps[:], b_t[:, jt : jt + 1]
        )

    def proj_v(st_):
        """one s-tile of vh: out partition=s, free=[4 heads x 64]; masked rows zeroed."""
        ps = ps_mm.tile([128, HC], f32, tag="mm")
        for k in range(KCH):
            nc.tensor.matmul(
                ps[:],
                xts[:, k, st_ * 128 : (st_ + 1) * 128],
                wv_t[:, k, :],
                start=(k == 0),
                stop=(k == KCH - 1),
            )
            if k % 2 == 1:
                yield
        vsl = vh_t[:, st_, :, 0:HD]
        nc.vector.tensor_tensor(
            vsl,
            ps[:].rearrange("p (h d) -> p h d", h=HPC),
            bvb[:].rearrange("p (h d) -> p h d", h=HPC),
            ADD,
        )
        nc.vector.tensor_scalar_mul(vsl, vsl, padv_t[:, st_ : st_ + 1])

    agsb_tiles = {}
    den_tiles = {}

    def norm_recip(qi):
        # one batched reciprocal of all 16 gathered denominators
        den_sb = den_tiles[qi]
        den_f = stg_pool.tile([H, QT], f32, tag="denf")
        nc.vector.tensor_copy(den_f[:], den_sb[:])
        rec_f = stg_pool.tile([H, QT], f32, tag="recf")
        nc.vector.reciprocal(rec_f[:], den_f[:])
        rec16 = rec16s[qi % 2]
        with nc.allow_low_precision(reason="bf16 staging precision"):
            nc.vector.tensor_copy(rec16[0:H, :], rec_f[:])
        yield

    def norm_chunks(qi, cs):
        # broadcast 1/den per head over its 64 rows; scale agsb in place
        agsb = agsb_tiles[qi]
        rec16 = rec16s[qi % 2]
        for c in cs:
            bcst_ps = ps_mm.tile([128, QT], f32, tag="mm")
            nc.tensor.matmul(
                bcst_ps[:], esel_t[:, c, :], rec16[:], start=True, stop=True
            )
            with nc.allow_low_precision(reason="bf16 staging precision"):
                nc.vector.tensor_tensor(
                    agsb[:, c, :], agsb[:, c, :], bcst_ps[:], MULT
                )
            yield

    def o_chunk(qi, jh):
        """O-projection for q-tile qi, output-column half jh (128 j cols)."""
        agsb = agsb_tiles[qi]
        ps = ps_mm.tile([128, QT], f32, tag="mm")
        for dc in range(KCH):
            nc.tensor.matmul(
                ps[:],
                wo_t[:, dc, jh * 128 : (jh + 1) * 128],
                agsb[:, dc, :],
                start=(dc == 0),
                stop=(dc == KCH - 1),
            )
            if dc % 2 == 1:
                yield
        osb = outsb_pool.tile([128, QT], f32, tag="osb")
        nc.vector.tensor_scalar_add(osb[:], ps[:], bo_t[:, jh : jh + 1])
        nc.sync.dma_start(
            outT[jh * 128 : (jh + 1) * 128, qi * QT : (qi + 1) * QT], osb[:]
        )

    def attention_qtile(qi, filler):
        q0 = qi * QT
        nk = nk_cs[qi]
        bk0 = bk0s[qi]
        for pair in range(2):
            probs = probs_pool.tile([128, NKT_C, 2, QT], bf16, tag="probs")

            def emit_sc(kt):
                k0 = kt * 128
                st = ps_st.tile([128, 2, QT], f32, tag="st")
                for hh in range(2):
                    nc.tensor.matmul(
                        st[:, hh, :],
                        kh_t[hh * 64 : hh * 64 + 64, pair, k0 : k0 + 128],
                        qh_t[hh * 64 : hh * 64 + 64, pair, q0 : q0 + QT],
                        start=True,
                        stop=True,
                    )
                if kt >= bk0:  # causal staircase mask (host-precomputed)
                    nc.vector.tensor_tensor(
                        st[:, :, :],
                        st[:, :, :],
                        km_t[:, qi, kt - bk0, :].rearrange(
                            "p (o n) -> p o n", o=1
                        ).broadcast_to([128, 2, QT]),
                        ADD,
                    )
                nc.scalar.activation(
                    probs[:, kt, :, :],
                    st[:, :, :],
                    EXP,
                    bias=0.0,
                    scale=float(SCALE),
                )

            emit_sc(0)
            ot0 = ps_ot.tile([HD + 1, QT], f32, tag="ot")
            ot1 = ps_ot.tile([HD + 1, QT], f32, tag="ot")
            ots = (ot0, ot1)
            for kt in range(nk):
                if kt + 1 < nk:
                    emit_sc(kt + 1)
                for hh in range(2):
                    h = 2 * pair + hh
                    nc.tensor.matmul(
                        ots[hh][:, :],
                        vh_t[:, kt, h, :],
                        probs[:, kt, hh, :],
                        start=(kt == 0),
                        stop=(kt == nk - 1),
                    )
                filler()
            # stage unnormalized o (rows 0-255) + denominator (rows 256-259)
            for hh in range(2):
                h = 2 * pair + hh
                stg = stg_pool.tile([HD + 1, QT], bf16, tag="stg")
                nc.vector.tensor_copy(stg[:], ots[hh][:])
                nc.sync.dma_start(
                    ag_in[qi, h * HD : (h + 1) * HD, :], stg[0:HD, :]
                )
                nc.sync.dma_start(
                    ag_in[qi, HPC * HD + h : HPC * HD + h + 1, :], stg[HD : HD + 1, :]
                )

    # ---- emission: projections + O-chunks finely interleaved -------------
    def kneed(qi):
        return min(NCK, (nk_cs[qi] * 128 + QT - 1) // QT)

    emitted = {"kc": 0, "qc": 0, "vst": 0}

    def proj_units_for(qi):
        # units that must complete before attention(qi) runs
        units = []
        for c in range(emitted["kc"], kneed(qi)):
            for jt in range(2):
                units.append(
                    lambda jt=jt, c=c: proj_qk(wk_t, bk_t, kh_t, jt, c, xts)
                )
        emitted["kc"] = max(emitted["kc"], kneed(qi))
        for c in range(emitted["qc"], qi + 1):
            for jt in range(2):
                units.append(
                    lambda jt=jt, c=c: proj_qk(wq_t, bq_t, qh_t, jt, c, xt)
                )
        emitted["qc"] = max(emitted["qc"], qi + 1)
        for st_ in range(emitted["vst"], nk_cs[qi]):
            units.append(lambda st_=st_: proj_v(st_))
        emitted["vst"] = max(emitted["vst"], nk_cs[qi])
        return units

    def o_chunk_units(qi):
        return (
            [lambda qi=qi: norm_recip(qi)]
            + [
                lambda qi=qi, cs=cs: norm_chunks(qi, cs)
                for cs in ((0, 1, 2, 3), (4, 5, 6, 7))
            ]
            + [lambda jh=jh, qi=qi: o_chunk(qi, jh) for jh in range(2)]
        )

    class Filler:
        def __init__(self, units, budget):
            self.units = list(units)
            self.gen = None
            self.budget = budget

        def __call__(self):
            for _ in range(self.budget):
                if self.gen is None:
                    if not self.units:
                        return
                    self.gen = self.units.pop(0)()
                try:
                    next(self.gen)
                except StopIteration:
                    self.gen = None

        def flush(self):
            while self.units or self.gen is not None:
                if self.gen is None:
                    self.gen = self.units.pop(0)()
                for _ in self.gen:
                    pass
                self.gen = None

    Filler(proj_units_for(0), 1).flush()
    for qi in range(NQT):
        pending = proj_units_for(qi + 1) if qi + 1 < NQT else []
        n_att = 2 * nk_cs[qi]             # filler() call sites this q-tile
        total_steps = len(pending) * 5
        budget = max(1, (total_steps + n_att - 1) // n_att)
        filler = Filler(pending, budget)
        attention_qtile(qi, filler)
        filler.flush()
        # AllGather this q-tile's head outputs across the group
        nc.gpsimd.collective_compute(
            "AllGather",
            mybir.AluOpType.bypass,
            replica_groups=REPLICA_GROUPS,
            ins=[ag_in[qi]],
            outs=[ag_out[qi]],
        )
        agsb = agsb_pool.tile([128, KCH, QT], bf16, tag="agsb")
        agsb_tiles[qi] = agsb
        den_sb = agsb_pool.tile([H, QT], bf16, tag="den")
        den_tiles[qi] = den_sb
        RB = HPC * HR  # 260 rows per rank block
        for r in range(GROUP):
            nc.gpsimd.dma_start(
                agsb[:, 2 * r : 2 * r + 2, :],
                ag_out[qi, r * RB : r * RB + HPC * HD, :].rearrange(
                    "(c p) q -> p c q", p=128
                ),
            )
            nc.gpsimd.dma_start(
                den_sb[HPC * r : HPC * r + HPC, :],
                ag_out[qi, r * RB + HPC * HD : (r + 1) * RB, :],
            )
        # normalize + O-project q-tile qi-2 (its AllGather finished two
        # attention spans ago) while AG(qi) is on the wire
        if qi >= 2:
            Filler(o_chunk_units(qi - 2), 1).flush()

    # tail: O-chunk for q-tile 2, keep the PE clock warm while the last
    # AllGather drains, then the final O-chunk
    Filler(o_chunk_units(NQT - 2), 1).flush()
    warm_mms(110)
    Filler(o_chunk_units(NQT - 1), 1).flush()

    for p in reversed(ctx_pools):
        p.__exit__(None, None, None)


# ---- host-side marshalling ----------------------------------------------


def compact_cfg(pad_mask):
    """Key-compaction geometry shared by both batches (max-padded)."""
    pad_mask = np.asarray(pad_mask)
    sels = [np.where(~pad_mask[b])[0] for b in range(B)]

    def cnt(b, p):
        return int(np.searchsorted(sels[b], p))

    nk_cs, bk0s = [], []
    for qi in range(NQT):
        q0 = qi * QT
        c_end = max(cnt(b, q0 + QT) for b in range(B))
        nk_cs.append(max(1, -(-c_end // 128)))
        bk0s.append(min(cnt(b, q0) // 128 for b in range(B)))
    return {
        "NKT_C": nk_cs[-1],
        "NCK": -(-(nk_cs[-1] * 128) // QT),
        "nk_cs": nk_cs,
        "bk0s": bk0s,
        "MAXBD": max(nk_cs[i] - bk0s[i] for i in range(NQT)),
        "sels": sels,
    }


def make_inputs(q, pad_mask, Wq, bq, Wk, bk, Wv, bv, Wo, bo):
    """Build the 8 per-core input maps from full inputs."""
    bf = ml_dtypes.bfloat16
    cfg = compact_cfg(pad_mask)
    NKT_C, NCK, MAXBD = cfg["NKT_C"], cfg["NCK"], cfg["MAXBD"]
    SK = NCK * QT
    esel_m = np.ascontiguousarray(
        (np.arange(D)[None, :] // HD == np.arange(H)[:, None]).astype(bf)
    )
    in_maps = []
    xTs_full = [np.ascontiguousarray(q[b].T).astype(bf) for b in range(B)]
    xTs_sel, padvs, kmasks = [], [], []
    for b in range(B):
        sel = cfg["sels"][b]
        n_sel = len(sel)
        xs = np.zeros((SK, D), dtype=np.float32)
        xs[:n_sel] = np.asarray(q[b])[sel]
        xTs_sel.append(np.ascontiguousarray(xs.T).astype(bf))
        padvs.append(
            np.ascontiguousarray(
                (np.arange(NKT_C * 128) < n_sel)
                .astype(np.float32)
                .reshape(NKT_C, 128)
                .T
            )
        )
        km = np.zeros((NQT, MAXBD, 128, QT), dtype=np.float32)
        for qi in range(NQT):
            qpos = qi * QT + np.arange(QT)
            for j in range(MAXBD):
                kt = cfg["bk0s"][qi] + j
                if kt >= cfg["nk_cs"][qi]:
                    continue
                idx = kt * 128 + np.arange(128)
                valid = idx < n_sel
                pos = np.where(valid, sel[np.minimum(idx, n_sel - 1)], -1)
                km[qi, j] = np.where(
                    valid[:, None] & (pos[:, None] > qpos[None, :]),
                    np.float32(NEG),
                    np.float32(0),
                )
        kmasks.append(np.ascontiguousarray(km.astype(bf)))
    for core in range(8):
        b, r = divmod(core, GROUP)
        sl = slice(r * HC, (r + 1) * HC)
        in_maps.append(
            {
                "xT": xTs_full[b],
                "xTs": xTs_sel[b],
                "wqT": np.ascontiguousarray(Wq[sl, :].T).astype(bf),
                "wkT": np.ascontiguousarray(Wk[sl, :].T).astype(bf),
                "wvT": np.ascontiguousarray(Wv[sl, :].T).astype(bf),
                "woT": np.ascontiguousarray(Wo[sl, :].T).astype(bf),
                "bq": np.ascontiguousarray(bq[sl].reshape(2, 128).T).astype(np.float32),
                "bk": np.ascontiguousarray(bk[sl].reshape(2, 128).T).astype(np.float32),
                "bv": np.ascontiguousarray(bv[sl].reshape(1, HC)).astype(bf),
                "bo": np.ascontiguousarray(bo[sl].reshape(2, 128).T).astype(np.float32),
                "padv": padvs[b],
                "kmask": kmasks[b],
                "esel": esel_m,
            }
        )
    return in_maps


def assemble_output(results):
    full = np.empty((B, S, D), dtype=np.float32)
    for core in range(8):
        b, r = divmod(core, GROUP)
        full[b, :, r * HC : (r + 1) * HC] = results[core]["outT"].T
    return full


_NC_CACHE = [None]
_CFG_KEY = [None]


def kernel(**inputs):
    """Full-input MHA forward. inputs: q, pad_mask, Wq, bq, Wk, bk, Wv, bv,
    Wo, bo (as produced by setup_inputs). Returns [B, S, D] float32."""
    inputs = {k: np.asarray(v) for k, v in inputs.items()}
    mask_key = inputs["pad_mask"].tobytes()
    if _NC_CACHE[0] is None or _CFG_KEY[0] != mask_key:
        _NC_CACHE[0] = build(compact_cfg(inputs["pad_mask"]))
        _CFG_KEY[0] = mask_key
    nc = _NC_CACHE[0]
    in_maps = make_inputs(**inputs)
    res = run_bass_kernel_spmd(nc, in_maps, list(range(8)))
    return assemble_output(res.results)


# revision 30
# speedup vs baseline: 1.0962x; 1.0070x over previous
"""Multi-head attention (B=2, S=2048, D=1024, H=16, causal + key-pad mask)
as an 8-core Trainium2 Bass/Tile SPMD kernel.

Sharding: data parallel over the 2 batches (4 cores each); within a batch
group, tensor parallel over heads (4 heads/core) for the QKV projections and
attention. Head outputs are softmax-normalized on the owning core, cast to
bf16 and AllGathered per 512-wide q-tile; the O-projection is column-sliced
(each core computes its own 256 output columns for ALL rows) and pipelined
per q-tile into the attention stream, so only the last AllGather plus one
small O-chunk sits on the serial tail.

All matmul operands are bf16 (fp32 PSUM accumulation): bf16 streams at
1 cyc/row on the PE where f32r measured ~2.7, and it halves HBM/SBUF/wire
traffic. The key-pad mask is folded into the V projection (masked key rows
and their denominator 'ones' column are zeroed), so exp needs no bias.
Causal masking: block-level loop bounds + a -1e9 triangular DVE add on
diagonal blocks. Warm-up and keep-alive matmuls prevent the PE HAM clock
from dropping to half rate during DMA/collective-only windows.

self-contained: includes a workaround for the walrus per-instruction
sync-wait limit and an NTFF-profile hook shim.
"""
import sys
import types

import numpy as np
import ml_dtypes

import bass_rust
import concourse.bass as bass
import concourse.mybir as mybir
import concourse.tile as tile


# ---- walrus sync-wait limit workaround ----------------------------------
# This walrus build rejects instructions carrying more than one sem wait
# ("Too many sync wait commands"). Tile emits multi-wait instructions (the
# final drain, matmuls waiting on several DMA queues). Split excess waits
# onto same-engine NoOps placed immediately before the instruction --
# serial waits on one sequencer are semantically identical.
_WSPLIT_COUNTER = [0]


def _split_excess_waits(nc, limit=1):
    for fn in nc.m.functions:
        for bb in fn.blocks:
            out = []
            changed = False
            for inst in bb.instructions:
                si = inst.sync_info
                waits = list(si.on_wait) if si is not None and si.on_wait else []
                if len(waits) > limit:
                    extra, keep = waits[:-limit], waits[-limit:]
                    for s in range(0, len(extra), limit):
                        _WSPLIT_COUNTER[0] += 1
                        nop = mybir.InstNoOp(
                            name=f"I-wsplit-{_WSPLIT_COUNTER[0]}", ins=[], outs=[]
                        )
                        nop.engine = inst.engine
                        nop.sync_info = bass_rust.SyncInfo(
                            on_wait=extra[s : s + limit], on_update=[]
                        )
                        out.append(nop)
                    si.on_wait = keep
                    changed = True
                out.append(inst)
            if changed:
                bb.instructions = out


def _install_tile_patch():
    if getattr(tile.TileContext, "_wait_split_patched", False):
        return
    orig_exit = tile.TileContext.__exit__

    def __exit__(self, exc_type, exc_val, exc_tb):
        r = orig_exit(self, exc_type, exc_val, exc_tb)
        if exc_type is None:
            _split_excess_waits(self.nc)
        return r

    tile.TileContext.__exit__ = __exit__
    tile.TileContext._wait_split_patched = True


_install_tile_patch()


# ---- NTFF profile hook shim (axon deployments missing antenv.axon_hooks) --
def _install_ntff_hook():
    try:
        import antenv.axon_hooks  # noqa: F401
        return
    except ImportError:
        pass
    try:
        from trn_agent_boot.trn_boot import _ntff_profile_via_ctypes

        hook = _ntff_profile_via_ctypes("/opt/axon/libaxon_pjrt.so")
    except Exception:
        hook = None
    m = types.ModuleType("antenv.axon_hooks")
    m.get_axon_ntff_profile_hook = lambda: hook
    m.set_axon_ntff_profile_hook = lambda h: None
    sys.modules["antenv.axon_hooks"] = m


_install_ntff_hook()

from concourse.bass_utils import run_bass_kernel_spmd  # noqa: E402

f32 = mybir.dt.float32
f32r = mybir.dt.float32r
bf16 = mybir.dt.bfloat16

B, S, D, H, HD = 2, 2048, 1024, 16, 64
HPC, GROUP = 4, 4          # heads per core, cores per batch
HC = HPC * HD              # 256 projection cols per core
NKT = S // 128             # 16 k-tiles
NQT = S // 512             # 4 q-tiles
QT = 512                   # q-tile width
SCALE = 1.0 / np.sqrt(HD)  # 0.125
NEG = -1.0e9
KCH = D // 128             # 8 contraction chunks

REPLICA_GROUPS = [[0, 1, 2, 3], [4, 5, 6, 7]]

ADD = mybir.AluOpType.add
MULT = mybir.AluOpType.mult
EXP = mybir.ActivationFunctionType.Exp


def r32(ap):
    return ap.bitcast(f32r)


def build(cfg):
    # cfg: dict with NKT_C (compacted k-tiles), NCK (k-proj 512-col units),
    # nk_cs[NQT], bk0s[NQT], MAXBD (mask tiles per q-tile)
    nc = bass.Bass()
    dp = nc.declare_dram_parameter
    SK = cfg["NCK"] * QT
    xT = dp("xT", [D, S], bf16, isOutput=False)
    xTs = dp("xTs", [D, SK], bf16, isOutput=False)     # compacted keys (padded)
    wqT = dp("wqT", [D, HC], bf16, isOutput=False)
    wkT = dp("wkT", [D, HC], bf16, isOutput=False)
    wvT = dp("wvT", [D, HC], bf16, isOutput=False)
    woT = dp("woT", [D, HC], bf16, isOutput=False)     # this core's 256 out cols
    bq = dp("bq", [128, 2], f32, isOutput=False)
    bk = dp("bk", [128, 2], f32, isOutput=False)
    bv = dp("bv", [1, HC], bf16, isOutput=False)
    bo = dp("bo", [128, 2], f32, isOutput=False)       # this core's 256 out cols
    padv = dp("padv", [128, cfg["NKT_C"]], f32, isOutput=False)  # 1 real / 0 pad
    kmask = dp("kmask", [NQT, cfg["MAXBD"], 128, QT], bf16, isOutput=False)
    esel = dp("esel", [H, D], bf16, isOutput=False)     # head-of-column selector
    outT = dp("outT", [HC, S], f32, isOutput=True)

    with tile.TileContext(nc) as tc:
        _body(nc, tc, cfg, locals())
    return nc


def _body(nc, tc, cfg, t):
    xT, wqT, wkT, wvT, woT = t["xT"], t["wqT"], t["wkT"], t["wvT"], t["woT"]
    bq, bk, bv, bo, padv = t["bq"], t["bk"], t["bv"], t["bo"], t["padv"]
    xTs, kmask = t["xTs"], t["kmask"]
    esel = t["esel"]
    outT = t["outT"]
    NKT_C, NCK = cfg["NKT_C"], cfg["NCK"]
    nk_cs, bk0s, MAXBD = cfg["nk_cs"], cfg["bk0s"], cfg["MAXBD"]
    SK = NCK * QT

    ctx_pools = []

    def pool(name, bufs, space="SBUF"):
        p = tc.tile_pool(name=name, bufs=bufs, space=space)
        ctx_pools.append(p)
        return p.__enter__()

    dram_pool = pool("dram", 1, space="DRAM")
    HR = HD + 1  # 65 rows/head: 64 o-rows + softmax denominator
    ag_in = dram_pool.tile([NQT, HPC * HR, QT], bf16)       # [4, 260, 512]
    ag_out = dram_pool.tile([NQT, H * HR, QT], bf16)        # [4, 1040, 512]
    agw_in = dram_pool.tile([64, 64], bf16)
    agw_out = dram_pool.tile([GROUP * 64, 64], bf16)

    const = pool("const", 1)
    probs_pool = pool("probs", 2)
    agsb_pool = pool("agsb", 2)
    stg_pool = pool("stg", 3)
    outsb_pool = pool("outsb", 2)

    ps_st = pool("ps_st", 2, space="PSUM")
    ps_ot = pool("ps_ot", 2, space="PSUM")
    ps_mm = pool("ps_mm", 2, space="PSUM")

    # ---- PE warm-up: ~4us of dependency-free matmuls so the HAM clock ----
    # gate opens while the input DMAs stream in.
    wu_a = const.tile([128, 128], bf16)
    nc.any.memset(wu_a[:], 0.015625)
    wu_b = const.tile([128, QT], bf16)
    nc.any.memset(wu_b[:], 0.015625)

    def warm_mms(n):
        for _ in range(n):
            ps = ps_mm.tile([128, QT], f32, tag="mm")
            nc.tensor.matmul(ps[:], wu_a[:], wu_b[:], start=True, stop=True)

    warm_mms(40)

    # ---- resident inputs -------------------------------------------------
    xt = const.tile([128, KCH, S], bf16)      # xT, chunk-major (queries)
    xts = const.tile([128, KCH, SK], bf16)    # compacted keys, chunk-major
    wq_t = const.tile([128, KCH, HC], bf16)
    wk_t = const.tile([128, KCH, HC], bf16)
    wv_t = const.tile([128, KCH, HC], bf16)
    wo_t = const.tile([128, KCH, HC], bf16)
    bq_t = const.tile([128, 2], f32)
    nc.sync.dma_start(bq_t[:], bq[:])
    bk_t = const.tile([128, 2], f32)
    nc.sync.dma_start(bk_t[:], bk[:])
    bo_t = const.tile([128, 2], f32)
    nc.sync.dma_start(bo_t[:], bo[:])
    padv_t = const.tile([128, NKT_C], f32)
    nc.sync.dma_start(padv_t[:], padv[:])
    km_t = const.tile([128, NQT, MAXBD, QT], bf16)
    for n in range(NQT):
        for m in range(MAXBD):
            nc.scalar.dma_start(km_t[:, n, m, :], kmask[n, m])
    bv_row = const.tile([128, HC], bf16)
    nc.any.memset(bv_row[:], 0.0)
    nc.sync.dma_start(bv_row[0:1, :], bv[:])

    wqT_r = wqT.rearrange("(c p) j -> p c j", p=128)
    wkT_r = wkT.rearrange("(c p) j -> p c j", p=128)
    wvT_r = wvT.rearrange("(c p) j -> p c j", p=128)
    woT_r = woT.rearrange("(c p) j -> p c j", p=128)
    xT_r = xT.rearrange("(c p) s -> p c s", p=128)
    xTs_r = xTs.rearrange("(c p) s -> p c s", p=128)
    # critical path (q-tile 0) on the sync queue; the rest on gpsimd's queue
    for k in range(KCH):
        nc.sync.dma_start(wk_t[:, k], wkT_r[:, k])
        nc.sync.dma_start(wq_t[:, k], wqT_r[:, k])
        nc.sync.dma_start(xt[:, k, 0:QT], xT_r[:, k, 0:QT])
        nc.sync.dma_start(xts[:, k, 0:QT], xTs_r[:, k, 0:QT])
        nc.sync.dma_start(wv_t[:, k], wvT_r[:, k])
    # warm-up AllGather first on the gpsimd queue: absorbs the TOPSP cold
    # start during the input DMA phase so the first real AllGather runs fast
    nc.gpsimd.dma_start(agw_in[:], wu_a[0:64, 0:64])
    nc.gpsimd.collective_compute(
        "AllGather",
        mybir.AluOpType.bypass,
        replica_groups=REPLICA_GROUPS,
        ins=[agw_in[:]],
        outs=[agw_out[:]],
    )
    for c in range(1, NQT):
        for k in range(KCH):
            nc.sync.dma_start(
                xt[:, k, c * QT : (c + 1) * QT], xT_r[:, k, c * QT : (c + 1) * QT]
            )
    for c in range(1, NCK):
        for k in range(KCH):
            nc.sync.dma_start(
                xts[:, k, c * QT : (c + 1) * QT], xTs_r[:, k, c * QT : (c + 1) * QT]
            )
    for k in range(KCH):
        nc.gpsimd.dma_start(wo_t[:, k], woT_r[:, k])

    # e0: row 0 = ones, rest 0 -- broadcast-matmul stationary
    e0h = const.tile([128, 128], bf16)
    nc.any.memset(e0h[:], 0.0)
    ones_row_h = const.tile([1, 128], bf16)
    nc.any.memset(ones_row_h[:], 1.0)
    nc.vector.tensor_copy(e0h[0:1, :], ones_row_h[0:1, :])

    # bvb: bv broadcast to all 128 partitions (e0h row 0 is ones, rest 0)
    bvb = const.tile([128, HC], f32)
    bv_ps = ps_mm.tile([128, HC], f32, tag="mm")
    nc.tensor.matmul(bv_ps[:], e0h[:], bv_row[:], start=True, stop=True)
    nc.vector.tensor_copy(bvb[:], bv_ps[:])

    # esel stationary (rows 16-127 zero) and reciprocal-broadcast tiles
    esel_t = const.tile([128, KCH, 128], bf16)
    nc.any.memset(esel_t[:], 0.0)
    nc.sync.dma_start(
        esel_t[0:H], esel.rearrange("h (c n) -> h c n", n=128)
    )
    rec16_a = const.tile([128, QT], bf16)
    nc.any.memset(rec16_a[:], 0.0)
    rec16_b = const.tile([128, QT], bf16)
    nc.any.memset(rec16_b[:], 0.0)
    rec16s = (rec16_a, rec16_b)

    # projection outputs
    qh_t = const.tile([128, 2, S], bf16)    # [j-in-tile, j-tile(pair), s]
    kh_t = const.tile([128, 2, SK], bf16)
    vh_t = const.tile([128, NKT_C, HPC, HD + 1], bf16)
    # denominator 'ones' column: 1.0 for real keys, 0.0 for padding
    nc.vector.tensor_copy(
        vh_t[:, :, :, HD : HD + 1],
        padv_t[:].rearrange("p (t u v) -> p t u v", u=1, v=1).broadcast_to(
            [128, NKT_C, HPC, 1]
        ),
    )

    def proj_qk(w_t, b_t, out_t, jt, c, x_src):
        # one [128, 512] tile of qhT/khT: out partition=j, free=s
        ps = ps_mm.tile([128, QT], f32, tag="mm")
        for k in range(KCH):
            nc.tensor.matmul(
                ps[:],
                w_t[:, k, jt * 128 : (jt + 1) * 128],
                x_src[:, k, c * QT : (c + 1) * QT],
                start=(k == 0),
                stop=(k == KCH - 1),
            )
            if k % 2 == 1:
                yield
        nc.vector.tensor_scalar_add(
            out_t[:, jt, c * QT : (c + 1) * QT], ps[:], b_t[:, jt : jt + 1]
        )

    def proj_v(st_):
        """one s-tile of vh: out partition=s, free=[4 heads x 64]; masked rows zeroed."""
        ps = ps_mm.tile([128, HC], f32, tag="mm")
        for k in range(KCH):
            nc.tensor.matmul(
                ps[:],
                xts[:, k, st_ * 128 : (st_ + 1) * 128],
                wv_t[:, k, :],
                start=(k == 0),
                stop=(k == KCH - 1),
            )
            if k % 2 == 1:
                yield
        vsl = vh_t[:, st_, :, 0:HD]
        nc.vector.tensor_tensor(
            vsl,
            ps[:].rearrange("p (h d) -> p h d", h=HPC),
            bvb[:].rearrange("p (h d) -> p h d", h=HPC),
            ADD,
        )
        nc.vector.tensor_scalar_mul(vsl, vsl, padv_t[:, st_ : st_ + 1])

    agsb_tiles = {}
    den_tiles = {}

    def norm_recip(qi):
        # one batched reciprocal of all 16 gathered denominators
        den_sb = den_tiles[qi]
        den_f = stg_pool.tile([H, QT], f32, tag="denf")
        nc.vector.tensor_copy(den_f[:], den_sb[:])
        rec_f = stg_pool.tile([H, QT], f32, tag="recf")
        nc.vector.reciprocal(rec_f[:], den_f[:])
        rec16 = rec16s[qi % 2]
        with nc.allow_low_precision(reason="bf16 staging precision"):
            nc.vector.tensor_copy(rec16[0:H, :], rec_f[:])
        yield

    def norm_chunks(qi, cs):
        # broadcast 1/den per head over its 64 rows; scale agsb in place
        agsb = agsb_tiles[qi]
        rec16 = rec16s[qi % 2]
        for c in cs:
            bcst_ps = ps_mm.tile([128, QT], f32, tag="mm")
            nc.tensor.matmul(
                bcst_ps[:], esel_t[:, c, :], rec16[:], start=True, stop=True
            )
            with nc.allow_low_precision(reason="bf16 staging precision"):
                nc.vector.tensor_tensor(
                    agsb[:, c, :], agsb[:, c, :], bcst_ps[:], MULT
                )
            yield

    def o_chunk(qi, jh):
        """O-projection for q-tile qi, output-column half jh (128 j cols)."""
        agsb = agsb_tiles[qi]
        ps = ps_mm.tile([128, QT], f32, tag="mm")
        for dc in range(KCH):
            nc.tensor.matmul(
                ps[:],
                wo_t[:, dc, jh * 128 : (jh + 1) * 128],
                agsb[:, dc, :],
                start=(dc == 0),
                stop=(dc == KCH - 1),
            )
            if dc % 2 == 1:
                yield
        osb = outsb_pool.tile([128, QT], f32, tag="osb")
        nc.vector.tensor_scalar_add(osb[:], ps[:], bo_t[:, jh : jh + 1])
        nc.sync.dma_start(
            outT[jh * 128 : (jh + 1) * 128, qi * QT : (qi + 1) * QT], osb[:]
        )

    def attention_qtile(qi, filler):
        q0 = qi * QT
        nk = nk_cs[qi]
        bk0 = bk0s[qi]
        for pair in range(2):
            probs = probs_pool.tile([128, NKT_C, 2, QT], bf16, tag="probs")

            def emit_sc(kt):
                k0 = kt * 128
                st = ps_st.tile([128, 2, QT], f32, tag="st")
                for hh in range(2):
                    nc.tensor.matmul(
                        st[:, hh, :],
                        kh_t[hh * 64 : hh * 64 + 64, pair, k0 : k0 + 128],
                        qh_t[hh * 64 : hh * 64 + 64, pair, q0 : q0 + QT],
                        start=True,
                        stop=True,
                    )
                if kt >= bk0:  # causal staircase mask (host-precomputed)
                    nc.vector.tensor_tensor(
                        st[:, :, :],
                        st[:, :, :],
                        km_t[:, qi, kt - bk0, :].rearrange(
                            "p (o n) -> p o n", o=1
                        ).broadcast_to([128, 2, QT]),
                        ADD,
                    )
                nc.scalar.activation(
                    probs[:, kt, :, :],
                    st[:, :, :],
                    EXP,
                    bias=0.0,
                    scale=float(SCALE),
                )

            emit_sc(0)
            ot0 = ps_ot.tile([HD + 1, QT], f32, tag="ot")
            ot1 = ps_ot.tile([HD + 1, QT], f32, tag="ot")
            ots = (ot0, ot1)
            for kt in range(nk):
                if kt + 1 < nk:
                    emit_sc(kt + 1)
                for hh in range(2):
                    h = 2 * pair + hh
                    nc.tensor.matmul(
                        ots[hh][:, :],
                        vh_t[:, kt, h, :],
                        probs[:, kt, hh, :],
                        start=(kt == 0),
                        stop=(kt == nk - 1),
                    )
                filler()
            # stage unnormalized o (rows 0-255) + denominator (rows 256-259)
            for hh in range(2):
                h = 2 * pair + hh
                stg = stg_pool.tile([HD + 1, QT], bf16, tag="stg")
                nc.vector.tensor_copy(stg[:], ots[hh][:])
                nc.sync.dma_start(
                    ag_in[qi, h * HD : (h + 1) * HD, :], stg[0:HD, :]
                )
                nc.sync.dma_start(
                    ag_in[qi, HPC * HD + h : HPC * HD + h + 1, :], stg[HD : HD + 1, :]
                )

    # ---- emission: projections + O-chunks finely interleaved -------------
    def kneed(qi):
        return min(NCK, (nk_cs[qi] * 128 + QT - 1) // QT)

    emitted = {"kc": 0, "qc": 0, "vst": 0}

    def proj_units_for(qi):
        # units that must complete before attention(qi) runs
        units = []
        for c in range(emitted["kc"], kneed(qi)):
            for jt in range(2):
                units.append(
                    lambda jt=jt, c=c: proj_qk(wk_t, bk_t, kh_t, jt, c, xts)
                )
        emitted["kc"] = max(emitted["kc"], kneed(qi))
        for c in range(emitted["qc"], qi + 1):
            for jt in range(2):
                units.append(
                    lambda jt=jt, c=c: proj_qk(wq_t, bq_t, qh_t, jt, c, xt)
                )
        emitted["qc"] = max(emitted["qc"], qi + 1)
        for st_ in range(emitted["vst"], nk_cs[qi]):
            units.append(lambda st_=st_: proj_v(st_))
        emitted["vst"] = max(emitted["vst"], nk_cs[qi])
        return units

    def o_chunk_units(qi):
        return (
            [lambda qi=qi: norm_recip(qi)]
            + [
                lambda qi=qi, cs=cs: norm_chunks(qi, cs)
                for cs in ((0, 1, 2, 3), (4, 5, 6, 7))
            ]
            + [lambda jh=jh, qi=qi: o_chunk(qi, jh) for jh in range(2)]
        )

    class Filler:
        def __init__(self, units, budget):
            self.units = list(units)
            self.gen = None
            self.budget = budget

        def __call__(self):
            for _ in range(self.budget):
                if self.gen is None:
                    if not self.units:
                        return
                    self.gen = self.units.pop(0)()
                try:
                    next(self.gen)
                except StopIteration:
                    self.gen = None

        def flush(self):
            while self.units or self.gen is not None:
                if self.gen is None:
                    self.gen = self.units.pop(0)()
                for _ in self.gen:
                    pass
                self.gen = None

    Filler(proj_units_for(0), 1).flush()
    for qi in range(NQT):
        pending = proj_units_for(qi + 1) if qi + 1 < NQT else []
        n_att = 2 * nk_cs[qi]             # filler() call sites this q-tile
        total_steps = len(pending) * 5
        budget = max(1, (total_steps + n_att - 1) // n_att)
        filler = Filler(pending, budget)
        attention_qtile(qi, filler)
        filler.flush()
        # AllGather this q-tile's head outputs across the group
        nc.gpsimd.collective_compute(
            "AllGather",
            mybir.AluOpType.bypass,
            replica_groups=REPLICA_GROUPS,
            ins=[ag_in[qi]],
            outs=[ag_out[qi]],
        )
        agsb = agsb_pool.tile([128, KCH, QT], bf16, tag="agsb")
        agsb_tiles[qi] = agsb
        den_sb = agsb_pool.tile([H, QT], bf16, tag="den")
        den_tiles[qi] = den_sb
        RB = HPC * HR  # 260 rows per rank block
        for r in range(GROUP):
            nc.gpsimd.dma_start(
                agsb[:, 2 * r : 2 * r + 2, :],
                ag_out[qi, r * RB : r * RB + HPC * HD, :].rearrange(
                    "(c p) q -> p c q", p=128
                ),
            )
            nc.gpsimd.dma_start(
                den_sb[HPC * r : HPC * r + HPC, :],
                ag_out[qi, r * RB + HPC * HD : (r + 1) * RB, :],
            )
        # normalize + O-project q-tile qi-2 (its AllGather finished two
        # attention spans ago) while AG(qi) is on the wire
        if qi >= 2:
            Filler(o_chunk_units(qi - 2), 1).flush()

    # tail: O-chunk for q-tile 2, keep the PE clock warm while the last
    # AllGather drains, then the final O-chunk
    Filler(o_chunk_units(NQT - 2), 1).flush()
    warm_mms(110)
    Filler(o_chunk_units(NQT - 1), 1).flush()

    for p in reversed(ctx_pools):
        p.__exit__(None, None, None)


# ---- host-side marshalling ----------------------------------------------


def compact_cfg(pad_mask):
    """Key-compaction geometry shared by both batches (max-padded)."""
    pad_mask = np.asarray(pad_mask)
    sels = [np.where(~pad_mask[b])[0] for b in range(B)]

    def cnt(b, p):
        return int(np.searchsorted(sels[b], p))

    nk_cs, bk0s = [], []
    for qi in range(NQT):
        q0 = qi * QT
        c_end = max(cnt(b, q0 + QT) for b in range(B))
        nk_cs.append(max(1, -(-c_end // 128)))
        bk0s.append(min(cnt(b, q0) // 128 for b in range(B)))
    return {
        "NKT_C": nk_cs[-1],
        "NCK": -(-(nk_cs[-1] * 128) // QT),
        "nk_cs": nk_cs,
        "bk0s": bk0s,
        "MAXBD": max(nk_cs[i] - bk0s[i] for i in range(NQT)),
        "sels": sels,
    }


def make_inputs(q, pad_mask, Wq, bq, Wk, bk, Wv, bv, Wo, bo):
    """Build the 8 per-core input maps from full inputs."""
    bf = ml_dtypes.bfloat16
    cfg = compact_cfg(pad_mask)
    NKT_C, NCK, MAXBD = cfg["NKT_C"], cfg["NCK"], cfg["MAXBD"]
    SK = NCK * QT
    esel_m = np.ascontiguousarray(
        (np.arange(D)[None, :] // HD == np.arange(H)[:, None]).astype(bf)
    )
    in_maps = []
    xTs_full = [np.ascontiguousarray(q[b].T).astype(bf) for b in range(B)]
    xTs_sel, padvs, kmasks = [], [], []
    for b in range(B):
        sel = cfg["sels"][b]
        n_sel = len(sel)
        xs = np.zeros((SK, D), dtype=np.float32)
        xs[:n_sel] = np.asarray(q[b])[sel]
        xTs_sel.append(np.ascontiguousarray(xs.T).astype(bf))
        padvs.append(
            np.ascontiguousarray(
                (np.arange(NKT_C * 128) < n_sel)
                .astype(np.float32)
                .reshape(NKT_C, 128)
                .T
            )
        )
        km = np.zeros((NQT, MAXBD, 128, QT), dtype=np.float32)
        for qi in range(NQT):
            qpos = qi * QT + np.arange(QT)
            for j in range(MAXBD):
                kt = cfg["bk0s"][qi] + j
                if kt >= cfg["nk_cs"][qi]:
                    continue
                idx = kt * 128 + np.arange(128)
                valid = idx < n_sel
                pos = np.where(valid, sel[np.minimum(idx, n_sel - 1)], -1)
                km[qi, j] = np.where(
                    valid[:, None] & (pos[:, None] > qpos[None, :]),
                    np.float32(NEG),
                    np.float32(0),
                )
        kmasks.append(np.ascontiguousarray(km.astype(bf)))
    for core in range(8):
        b, r = divmod(core, GROUP)
        sl = slice(r * HC, (r + 1) * HC)
        in_maps.append(
            {
                "xT": xTs_full[b],
                "xTs": xTs_sel[b],
                "wqT": np.ascontiguousarray(Wq[sl, :].T).astype(bf),
                "wkT": np.ascontiguousarray(Wk[sl, :].T).astype(bf),
                "wvT": np.ascontiguousarray(Wv[sl, :].T).astype(bf),
                "woT": np.ascontiguousarray(Wo[sl, :].T).astype(bf),
                "bq": np.ascontiguousarray(bq[sl].reshape(2, 128).T).astype(np.float32),
                "bk": np.ascontiguousarray(bk[sl].reshape(2, 128).T).astype(np.float32),
                "bv": np.ascontiguousarray(bv[sl].reshape(1, HC)).astype(bf),
                "bo": np.ascontiguousarray(bo[sl].reshape(2, 128).T).astype(np.float32),
                "padv": padvs[b],
                "kmask": kmasks[b],
                "esel": esel_m,
            }
        )
    return in_maps


def assemble_output(results):
    full = np.empty((B, S, D), dtype=np.float32)
    for core in range(8):
        b, r = divmod(core, GROUP)
        full[b, :, r * HC : (r + 1) * HC] = results[core]["outT"].T
    return full


_NC_CACHE = [None]
_CFG_KEY = [None]


def kernel(**inputs):
    """Full-input MHA forward. inputs: q, pad_mask, Wq, bq, Wk, bk, Wv, bv,
    Wo, bo (as produced by setup_inputs). Returns [B, S, D] float32."""
    inputs = {k: np.asarray(v) for k, v in inputs.items()}
    mask_key = inputs["pad_mask"].tobytes()
    if _NC_CACHE[0] is None or _CFG_KEY[0] != mask_key:
        _NC_CACHE[0] = build(compact_cfg(inputs["pad_mask"]))
        _CFG_KEY[0] = mask_key
    nc = _NC_CACHE[0]
    in_maps = make_inputs(**inputs)
    res = run_bass_kernel_spmd(nc, in_maps, list(range(8)))
    return assemble_output(res.results)
